# revision 1
# baseline (speedup 1.0000x reference)
"""Trainium2 Bass kernel for nn_DecoderLayer (dense transformer decoder layer).

Sharding: data-parallel over batch (4) x causal-balanced l-block split (2)
= 8 cores, no collectives.  Core (b, half) owns 4 l-blocks of 128 rows:
half 0 -> global blocks {0,3,4,7}, half 1 -> {1,2,5,6}; both sets have equal
causal attention work (18 block-units), served by ONE SPMD program with a
uniform per-slot suffix schedule [4,4,3,3,2,2,1,1] l-blocks (20 units, 11%
slack) and per-core mask data in {tri, zeros, ones} per kv slot.

All matmul operands are bf16 (fp32 PSUM accumulation); residual/LN in f32.

Faithful to the reference quirks:
  - q/k reshape is (head_dim, n_heads) interleaved -> per-head weight columns
    are strided slices, handled by host-side weight rearrangement.
  - the second attention einsum uses k (not v); v is never computed.
  - mask applied before scaling; softmax without max-subtraction is exact here
    because masked entries are exactly zeroed and scores are O(1).

Softmax row sums come free from the attn@k matmul: each head's ktil block
carries a ones column, putting sum_m e[m,l] in PSUM partition 64.
"""

import sys

sys.path.insert(0, "/opt/trn_rl_repo")

from contextlib import ExitStack

import numpy as np

import concourse.bass as bass  # noqa: F401  (registers types)
import concourse.mybir as mybir
import concourse.tile as tile
from concourse import bacc

f32 = mybir.dt.float32
DT = mybir.dt.bfloat16
F8 = mybir.dt.float8e4
FFN_SCALE = 32.0
AF = mybir.ActivationFunctionType
ALU = mybir.AluOpType

P = 128
B, L, D, M = 4, 1024, 1024, 1024
NH, HD, MLP = 16, 64, 4096
NPAIR = NH // 2          # 8 head pairs
FC = D // P              # 8 feature chunks
LLOC = L // 2            # 512 rows per core
LC = LLOC // P           # 4 l-chunks of 128
MMC = M // P             # 8 m-chunks (kv slots)
MLPC = MLP // P          # 32 mlp chunks
EPS = 1e-5
N_CORES = 8

BLK = ([0, 3, 4, 7], [1, 2, 5, 6])   # l-blocks per core half
# per-slot causal suffix schedule: slot j covers local l cols [SUF[j]*128, 512)
SUF = [0, 0, 1, 1, 2, 2, 3, 3]


def _ln_finish(nc, pool, h, stats, eps_t, g_bc, b_bc):
    """Finish LayerNorm from precomputed per-half bn_stats (aggr+apply)."""
    mv = pool.tile([P, 2], f32, tag="lnmv", name="lnmv")
    nc.vector.bn_aggr(out=mv[:], in_=stats)
    rstd = pool.tile([P, 1], f32, tag="lnr", name="lnr")
    nc.scalar.activation(out=rstd[:], in_=mv[:, 1:2], func=AF.Sqrt,
                         bias=eps_t[:], scale=1.0)
    nc.vector.reciprocal(out=rstd[:], in_=rstd[:])
    nc.vector.tensor_scalar(
        out=h[:], in0=h[:], scalar1=mv[:, 0:1], scalar2=rstd[:],
        op0=ALU.subtract, op1=ALU.mult,
    )
    if g_bc is not None:
        nc.vector.tensor_tensor(out=h[:], in0=h[:], in1=g_bc[:], op=ALU.mult)
    if b_bc is not None:
        nc.vector.tensor_tensor(out=h[:], in0=h[:], in1=b_bc[:], op=ALU.add)


def _build_program(ln_ident, repeat=1):
    """ln_ident: tuple of 3 bools -- gamma==1 and beta==0 for each LN."""
    nc = bacc.Bacc(None, target_bir_lowering=False)

    # ---- per-core inputs ----
    xk_d = nc.dram_tensor("xkT", [FC, P, M], DT, kind="ExternalInput")
    xq_d = nc.dram_tensor("xqT", [FC, P, LLOC], DT, kind="ExternalInput")
    enc_d = nc.dram_tensor("encT", [FC, P, M], DT, kind="ExternalInput")
    xr_d = nc.dram_tensor("xrows", [LC, P, D], DT, kind="ExternalInput")
    mka_d = nc.dram_tensor("maskA", [MMC, P, P], DT, kind="ExternalInput")
    mkb_d = nc.dram_tensor("maskB", [MMC, P, P], DT, kind="ExternalInput")
    # ---- shared inputs ----
    iddt_d = nc.dram_tensor("iddt", [P, P], DT, kind="ExternalInput")
    idf_d = nc.dram_tensor("idf32", [P, P], f32, kind="ExternalInput")
    sel_d = nc.dram_tensor("sel01", [33, P], DT, kind="ExternalInput")
    wq_s_d = nc.dram_tensor("wq_s", [NPAIR, P, FC, P], DT, kind="ExternalInput")
    wk_s_d = nc.dram_tensor("wk_s", [NPAIR, P, FC, P], DT, kind="ExternalInput")
    wo_s_d = nc.dram_tensor("wo_s", [NPAIR, P, D], DT, kind="ExternalInput")
    wq_c_d = nc.dram_tensor("wq_c", [NPAIR, P, FC, P], DT, kind="ExternalInput")
    wk_c_d = nc.dram_tensor("wk_c", [NPAIR, P, FC, P], DT, kind="ExternalInput")
    wo_c_d = nc.dram_tensor("wo_c", [NPAIR, P, D], DT, kind="ExternalInput")
    w1_d = nc.dram_tensor("ffw1", [MLPC, P, FC, P], DT, kind="ExternalInput")
    b1_d = nc.dram_tensor("ffb1", [P, MLPC], f32, kind="ExternalInput")
    w2_d = nc.dram_tensor("ffw2", [MLPC, P, D], DT, kind="ExternalInput")
    b2_d = nc.dram_tensor("ffb2", [P, D], f32, kind="ExternalInput")
    ln_bc_d = {}
    for i, ident in enumerate(ln_ident):
        if not ident:
            ln_bc_d[i] = (
                nc.dram_tensor(f"lng{i}", [P, D], f32, kind="ExternalInput"),
                nc.dram_tensor(f"lnb{i}", [P, D], f32, kind="ExternalInput"),
            )
    out_d = nc.dram_tensor("out", [LC, P, D], f32, kind="ExternalOutput")

    with tile.TileContext(nc) as tc:
        with ExitStack() as ctx:
            glob = ctx.enter_context(tc.tile_pool(name="glob", bufs=1))
            iddt = glob.tile([P, P], DT)
            nc.gpsimd.dma_start(out=iddt[:], in_=iddt_d[:])
            idf = glob.tile([P, P], f32)
            nc.gpsimd.dma_start(out=idf[:], in_=idf_d[:])
            sel01 = glob.tile([33, P], DT)
            nc.gpsimd.dma_start(out=sel01[:], in_=sel_d[:])
            eps_t = glob.tile([P, 1], f32)
            nc.vector.memset(eps_t[:], EPS)
            b2bc = glob.tile([P, D], f32)
            nc.gpsimd.dma_start(out=b2bc[:], in_=b2_d[:])
            b1t = glob.tile([P, MLPC], f32)
            nc.gpsimd.dma_start(out=b1t[:], in_=b1_d[:])
            ln_bc = {}
            for i, (g_d, b_d) in ln_bc_d.items():
                g_t = glob.tile([P, D], f32, name=f"lng{i}")
                nc.sync.dma_start(out=g_t[:], in_=g_d[:])
                b_t = glob.tile([P, D], f32, name=f"lnbt{i}")
                nc.sync.dma_start(out=b_t[:], in_=b_d[:])
                ln_bc[i] = (g_t, b_t)

            # activations / masks loaded once, reused across repeats
            xk = glob.tile([P, FC, M], DT, name="xk")
            nc.gpsimd.dma_start(out=xk[:], in_=xk_d.rearrange("f p m -> p f m"))
            xq = glob.tile([P, FC, LLOC], DT, name="xq")
            nc.gpsimd.dma_start(out=xq[:], in_=xq_d.rearrange("f p m -> p f m"))
            xros = glob.tile([P, LC, D], DT, name="xros")
            nc.gpsimd.dma_start(
                out=xros[:], in_=xr_d.rearrange("lc li d -> li lc d"))
            maskA = glob.tile([P, MMC, P], DT, name="maskA")
            nc.gpsimd.dma_start(
                out=maskA[:], in_=mka_d.rearrange("s pi l -> pi s l"))
            maskB = glob.tile([P, MMC, P], DT, name="maskB")
            nc.gpsimd.dma_start(
                out=maskB[:], in_=mkb_d.rearrange("s pi l -> pi s l"))
            enc = glob.tile([P, FC, M], DT, name="enc")
            nc.gpsimd.dma_start(out=enc[:], in_=enc_d.rearrange("f p m -> p f m"))

            def mha(sid, qsrc, kvsrc, wq_d, wk_d, wo_d, use_mask,
                    resid, h_tiles, hT_tile, ln_gb):
                """Attention block + residual + LN + transposed copy.

                qsrc(fo) -> (128, 512) DT AP; kvsrc(fo) -> (128, 1024) DT AP;
                resid(lc) -> (128, 1024) AP.  Writes h_tiles (4 x (128, 1024)
                f32 post-LN rows) and hT_tile ((128, FC, 512) DT).
                """
                with ExitStack() as SM:
                    opool = SM.enter_context(tc.tile_pool(name=f"om{sid}", bufs=1))
                    attno = opool.tile([P, NPAIR, LLOC], DT, name="attno")
                    # all-pair q/k projection weights, two half DMAs each
                    wqa = opool.tile([P, NPAIR, FC, P], DT, name="wqa")
                    wka = opool.tile([P, NPAIR, FC, P], DT, name="wka")
                    for hv in range(2):
                        nc.sync.dma_start(
                            out=wka[:, hv * 4:(hv + 1) * 4, :, :],
                            in_=wk_d[hv * 4:(hv + 1) * 4]
                            .rearrange("n p f c -> p n f c"))
                        nc.sync.dma_start(
                            out=wqa[:, hv * 4:(hv + 1) * 4, :, :],
                            in_=wq_d[hv * 4:(hv + 1) * 4]
                            .rearrange("n p f c -> p n f c"))
                    woa = opool.tile([P, NPAIR, D], DT, name="woa")
                    nc.sync.dma_start(out=woa[:],
                                      in_=wo_d.rearrange("n p d -> p n d"))
                    # ktil layout: [m, slot, head, 64 data | 1 ones | 63 zero]
                    ktil_pp = []
                    for i in range(2):
                        kt_i = opool.tile([P, MMC, 2, P], DT, name=f"ktil{i}")
                        nc.gpsimd.memset(kt_i[:, :, :, 64:128], 0.0)
                        nc.gpsimd.memset(kt_i[:, :, :, 64:65], 1.0)
                        ktil_pp.append(kt_i)

                    with ExitStack() as SAB:
                        psP = SAB.enter_context(
                            tc.tile_pool(name=f"psP{sid}", bufs=2, space="PSUM"))
                        psS = SAB.enter_context(
                            tc.tile_pool(name=f"psS{sid}", bufs=2, space="PSUM"))
                        kpool = SAB.enter_context(tc.tile_pool(name=f"k{sid}", bufs=2))
                        epool = SAB.enter_context(tc.tile_pool(name=f"e{sid}", bufs=3))
                        spool = SAB.enter_context(tc.tile_pool(name=f"s{sid}", bufs=2))

                        for p in range(NPAIR):
                            # kT for this pair: (128, 1024) over the full kv seq
                            kt = kpool.tile([P, M], DT, tag="kt", name="kt")
                            for half in range(2):
                                psk = psP.tile([P, 512], f32, tag="pp", name="psk")
                                for fo in range(FC):
                                    nc.tensor.matmul(
                                        psk[:], wka[:, p, fo, :],
                                        kvsrc(fo)[:, half * 512:(half + 1) * 512],
                                        start=(fo == 0), stop=(fo == FC - 1))
                                nc.vector.tensor_copy(
                                    out=kt[:, half * 512:(half + 1) * 512], in_=psk[:])

                            # k-tilde: per-slot (m, dA|dB) blocks via PE transpose
                            ktil = ktil_pp[p % 2]
                            for g in range(2):
                                pst = psP.tile([P, 512], DT, tag="pt", bufs=1,
                                               name="pst")
                                for m4 in range(4):
                                    slot = g * 4 + m4
                                    nc.tensor.transpose(
                                        pst[:, m4 * P:(m4 + 1) * P],
                                        kt[:, slot * P:(slot + 1) * P], iddt[:])
                                nc.vector.tensor_copy(
                                    out=ktil[:, g * 4:(g + 1) * 4, :, 0:64],
                                    in_=pst[:])

                            # qT for this pair: (128 = [dA|dB], 512)
                            psq = psP.tile([P, LLOC], f32, tag="pp", name="psq")
                            for fo in range(FC):
                                nc.tensor.matmul(psq[:], wqa[:, p, fo, :], qsrc(fo),
                                                 start=(fo == 0), stop=(fo == FC - 1))
                            qt = spool.tile([P, LLOC], DT, tag="qt", name="qt")
                            nc.vector.tensor_copy(out=qt[:], in_=psq[:])

                            # scores -> exp -> (mask) -> attn@k, scores one
                            # slot ahead so the PE never waits on ACT
                            pso = [psS.tile([P, LLOC], f32, tag="po", bufs=3,
                                            name=f"pso{hi}")
                                   for hi in range(2)]
                            e2 = {}
                            pend = []

                            def emit_scores(slot):
                                nv = (4 - SUF[slot]) * P if use_mask else LLOC
                                s0 = LLOC - nv
                                if slot % 2 == 0:
                                    for hi in range(2):
                                        e2[hi] = epool.tile(
                                            [P, 2, LLOC], DT, tag=f"e{hi}",
                                            name="e2")
                                for hi in range(2):
                                    bse = hi * 64
                                    pss = psS.tile([P, LLOC], f32, tag="ps",
                                                   bufs=2, name="pss")
                                    nc.tensor.matmul(
                                        pss[:, 0:nv],
                                        kt[bse:bse + 64, slot * P:(slot + 1) * P],
                                        qt[bse:bse + 64, s0:LLOC],
                                        start=True, stop=not use_mask)
                                    if use_mask:
                                        # additive -BIG mask folded into the
                                        # scores accumulation (rank-structured)
                                        nc.tensor.matmul(
                                            pss[:, 0:P],
                                            maskA[:, slot, :],
                                            maskB[:, slot, :],
                                            start=False, stop=True,
                                            skip_group_check=True)
                                    e_ap = e2[hi][:, slot % 2, :]
                                    nc.scalar.activation(out=e_ap[:, s0:LLOC],
                                                         in_=pss[:, 0:nv],
                                                         func=AF.Exp, scale=0.125)
                                    pend.append((slot, hi, e_ap, s0))

                            def emit_attnk():
                                slot, hi, e_ap, s0 = pend.pop(0)
                                nc.tensor.matmul(
                                    pso[hi][:, s0:LLOC],
                                    ktil[:, slot, hi, :],
                                    e_ap[:, s0:LLOC],
                                    start=(slot == 0), stop=(slot == MMC - 1))

                            for slot in range(MMC):
                                emit_scores(slot)
                                if slot >= 1:
                                    emit_attnk()
                                    emit_attnk()
                            emit_attnk()
                            emit_attnk()

                            # normalize: row sums sit at PSUM partition 64
                            srow = spool.tile([33, LLOC], DT, tag="sr", name="srow")
                            with nc.allow_low_precision(reason="softmax denom"):
                                for hi in range(2):
                                    nc.vector.reciprocal(
                                        out=srow[hi * 32:hi * 32 + 1, :],
                                        in_=pso[hi][64:65, :])
                            psbc = psP.tile([P, LLOC], f32, tag="pp", name="psbc")
                            for hi in range(2):
                                nc.tensor.matmul(
                                    psbc[:], sel01[hi * 32:hi * 32 + 1, :],
                                    srow[hi * 32:hi * 32 + 1, :],
                                    start=(hi == 0), stop=(hi == 1))
                            for hi in range(2):
                                nc.vector.tensor_copy(
                                    out=attno[hi * 64:hi * 64 + 64, p, :],
                                    in_=pso[hi][0:64, :])
                            nc.vector.tensor_tensor(
                                out=attno[:, p, :], in0=attno[:, p, :],
                                in1=psbc[:], op=ALU.mult)

                    # ---- output projection + residual + LN + transpose ----
                    with ExitStack() as SC:
                        psC = SC.enter_context(
                            tc.tile_pool(name=f"psC{sid}", bufs=2, space="PSUM"))
                        cpool = SC.enter_context(tc.tile_pool(name=f"c{sid}",
                                                              bufs=2))
                        lnst = cpool.tile([P, LC, 2, 6], f32, bufs=1,
                                          name="lnst")
                        psy = {}
                        for ng in range(2):
                            for lc in range(LC):
                                psy[lc] = psC.tile([P, 512], f32, tag=f"py{lc % 2}",
                                                   name=f"psy{lc}")
                            for p in range(NPAIR):
                                for lc in range(LC):
                                    nc.tensor.matmul(
                                        psy[lc][:],
                                        attno[:, p, lc * P:(lc + 1) * P],
                                        woa[:, p, ng * 512:(ng + 1) * 512],
                                        start=(p == 0),
                                        stop=(p == NPAIR - 1))
                            for lc in range(LC):
                                nc.vector.tensor_tensor(
                                    out=h_tiles[lc][:, ng * 512:(ng + 1) * 512],
                                    in0=psy[lc][:],
                                    in1=resid(lc)[:, ng * 512:(ng + 1) * 512],
                                    op=ALU.add)
                                nc.vector.bn_stats(
                                    out=lnst[:, lc, ng, :],
                                    in_=h_tiles[lc][:, ng * 512:(ng + 1) * 512])
                        g_bc, b_bc = ln_gb
                        for lc in range(LC):
                            h_t = h_tiles[lc]
                            _ln_finish(nc, cpool, h_t[:], lnst[:, lc, :, :],
                                       eps_t, g_bc, b_bc)
                            for g in range(2):
                                pst2 = psC.tile([P, 512], f32, tag="pt2",
                                                name="pst2")
                                for f4 in range(4):
                                    fo = g * 4 + f4
                                    nc.tensor.transpose(
                                        pst2[:, f4 * P:(f4 + 1) * P],
                                        h_t[:, fo * P:(fo + 1) * P], idf[:])
                                nc.vector.tensor_copy(
                                    out=hT_tile[:, g * 4:(g + 1) * 4,
                                                lc * P:(lc + 1) * P],
                                    in_=pst2[:])

            # ---- stage structure with pool lifetimes ----
            for _rep in range(repeat):
              with tc.tile_pool(name="h12", bufs=1) as h12pool:
                  h1 = [h12pool.tile([P, D], f32, name=f"h1_{lc}") for lc in range(LC)]
                  h1T = h12pool.tile([P, FC, LLOC], DT, name="h1T")

                  # stage 1: self-attention
                  mha(1,
                      qsrc=lambda fo: xq[:, fo, :],
                      kvsrc=lambda fo: xk[:, fo, :],
                      wq_d=wq_s_d, wk_d=wk_s_d, wo_d=wo_s_d,
                      use_mask=True,
                      resid=lambda lc: xros[:, lc, :],
                      h_tiles=h1, hT_tile=h1T,
                      ln_gb=ln_bc.get(0, (None, None)))

                  # stage 2: cross-attention (h2/h2T outlive this block)
                  with tc.tile_pool(name="h3p", bufs=1) as h3pool_outer:
                      h2 = [h3pool_outer.tile([P, D], f32, name=f"h2_{lc}")
                            for lc in range(LC)]
                      h2T = h3pool_outer.tile([P, FC, LLOC], DT, name="h2T")

                      mha(2,
                          qsrc=lambda fo: h1T[:, fo, :],
                          kvsrc=lambda fo: enc[:, fo, :],
                          wq_d=wq_c_d, wk_d=wk_c_d, wo_d=wo_c_d,
                          use_mask=False,
                          resid=lambda lc: h1[lc][:],
                          h_tiles=h2, hT_tile=h2T,
                          ln_gb=ln_bc.get(1, (None, None)))

                      # stage 3: FFN
                      with ExitStack() as s3:
                          ps3g = s3.enter_context(
                              tc.tile_pool(name="ps3g", bufs=2, space="PSUM"))
                          ps3y = s3.enter_context(
                              tc.tile_pool(name="ps3y", bufs=4, space="PSUM"))
                          wf = s3.enter_context(tc.tile_pool(name="wf", bufs=2))
                          gpool = s3.enter_context(tc.tile_pool(name="gp", bufs=1))
                          lpool = s3.enter_context(tc.tile_pool(name="lp", bufs=2))
                          gt = gpool.tile([P, MLPC, LLOC], DT, name="gt")
                          lnst3 = gpool.tile([P, LC, 2, 6], f32, name="lnst3")
                          h3 = [lpool.tile([P, D], f32, tag=f"h3_{lc % 2}",
                                           name=f"h3_{lc}") for lc in range(LC)]

                          for ng in range(2):
                              w2a = wf.tile([P, MLPC, 512], DT, tag="w2", bufs=1,
                                            name="w2a")
                              nc.sync.dma_start(
                                  out=w2a[:],
                                  in_=w2_d[:, :, ng * 512:(ng + 1) * 512]
                                  .rearrange("m p d -> p m d"))
                              psy2 = {}
                              for mc in range(MLPC):
                                  if ng == 0:
                                      if mc % 4 == 0:
                                          w1c = wf.tile([P, 4, FC, P], DT,
                                                        tag="w1", name="w1c")
                                          nc.sync.dma_start(
                                              out=w1c[:],
                                              in_=w1_d[mc:mc + 4]
                                              .rearrange("m p f c -> p m f c"))
                                      psg = ps3g.tile([P, LLOC], f32, tag="psg",
                                                      name="psg")
                                      for fo in range(FC):
                                          nc.tensor.matmul(
                                              psg[:], w1c[:, mc % 4, fo, :],
                                              h2T[:, fo, :],
                                              start=(fo == 0), stop=(fo == FC - 1))
                                      nc.scalar.activation(
                                          out=gt[:, mc, :], in_=psg[:], func=AF.Gelu,
                                          bias=b1t[:, mc:mc + 1], scale=1.0)
                                  if mc == 0:
                                      for lc in range(LC):
                                          psy2[lc] = ps3y.tile([P, 512], f32, tag="psy",
                                                               name=f"psy2_{lc}")
                                  for lc in range(LC):
                                      nc.tensor.matmul(
                                          psy2[lc][:], gt[:, mc, lc * P:(lc + 1) * P],
                                          w2a[:, mc, :], start=(mc == 0),
                                          stop=(mc == MLPC - 1))
                              for lc in range(LC):
                                  nc.vector.tensor_tensor(
                                      out=h3[lc][:, ng * 512:(ng + 1) * 512],
                                      in0=psy2[lc][:],
                                      in1=h2[lc][:, ng * 512:(ng + 1) * 512],
                                      op=ALU.add)
                                  nc.vector.tensor_tensor(
                                      out=h3[lc][:, ng * 512:(ng + 1) * 512],
                                      in0=h3[lc][:, ng * 512:(ng + 1) * 512],
                                      in1=b2bc[:, ng * 512:(ng + 1) * 512], op=ALU.add)
                                  nc.vector.bn_stats(
                                      out=lnst3[:, lc, ng, :],
                                      in_=h3[lc][:, ng * 512:(ng + 1) * 512])
                          g_bc, b_bc = ln_bc.get(2, (None, None))
                          for lc in range(LC):
                              _ln_finish(nc, lpool, h3[lc][:],
                                         lnst3[:, lc, :, :], eps_t, g_bc, b_bc)
                              nc.sync.dma_start(out=out_d[lc], in_=h3[lc][:])

    nc.finalize()
    return nc


# ---------------------------------------------------------------------------
# host side
# ---------------------------------------------------------------------------

_CACHE = {}


def _make_runner(nc, n_cores):
    import jax
    from jax.experimental.shard_map import shard_map
    from jax.sharding import Mesh, PartitionSpec
    from concourse.bass2jax import (_bass_exec_p, install_neuronx_cc_hook,
                                    partition_id_tensor)

    install_neuronx_cc_hook()
    partition_name = (nc.partition_id_tensor.name
                      if nc.partition_id_tensor else None)
    in_names, out_names, out_avals = [], [], []
    for alloc in nc.m.functions[0].allocations:
        if not isinstance(alloc, mybir.MemoryLocationSet):
            continue
        name = alloc.memorylocations[0].name
        if alloc.kind == "ExternalInput":
            if name != partition_name:
                in_names.append(name)
        elif alloc.kind == "ExternalOutput":
            out_names.append(name)
            out_avals.append(jax.core.ShapedArray(tuple(alloc.tensor_shape),
                                                  mybir.dt.np(alloc.dtype)))
    n_params = len(in_names)
    all_names = list(in_names) + list(out_names)
    if partition_name is not None:
        all_names.append(partition_name)

    def _body(*args):
        operands = list(args)
        if partition_name is not None:
            operands.append(partition_id_tensor())
        outs = _bass_exec_p.bind(
            *operands, out_avals=tuple(out_avals), in_names=tuple(all_names),
            out_names=tuple(out_names), lowering_input_output_aliases=(),
            sim_require_finite=True, sim_require_nnan=True, nc=nc)
        return tuple(outs)

    devices = jax.devices()[:n_cores]
    mesh = Mesh(np.asarray(devices), ("core",))
    n_outs = len(out_names)
    donate = tuple(range(n_params, n_params + n_outs))
    # inputs identical on every core are passed replicated (one transfer)
    per_core_names = {"xkT", "xqT", "encT", "xrows", "maskA", "maskB"}
    in_specs = tuple(
        PartitionSpec("core") if name in per_core_names else PartitionSpec()
        for name in in_names
    ) + (PartitionSpec("core"),) * n_outs
    sharded = jax.jit(
        shard_map(_body, mesh=mesh, in_specs=in_specs,
                  out_specs=(PartitionSpec("core"),) * n_outs,
                  check_rep=False),
        donate_argnums=donate, keep_unused=True)

    def pack(in_maps):
        args = []
        for name in in_names:
            if name in per_core_names:
                args.append(np.concatenate(
                    [np.asarray(in_maps[c][name]) for c in range(n_cores)],
                    axis=0))
            else:
                args.append(np.asarray(in_maps[0][name]))
        return args

    def unpack(out_arrs):
        out_arrs = [np.asarray(a) for a in out_arrs]
        return [
            {name: out_arrs[i].reshape(n_cores, *out_avals[i].shape)[c]
             for i, name in enumerate(out_names)}
            for c in range(n_cores)
        ]

    def fresh_zeros():
        return [np.zeros((n_cores * av.shape[0], *av.shape[1:]), av.dtype)
                for av in out_avals]

    def run(in_maps):
        out_arrs = sharded(*pack(in_maps), *fresh_zeros())
        return unpack(out_arrs)

    def timed_pipeline(in_maps, k=16):
        """Issue k executions asynchronously, block once; returns
        (results, total_seconds, k)."""
        import time
        from jax.sharding import NamedSharding
        args = pack(in_maps)
        dev_args = [jax.device_put(a, NamedSharding(mesh, in_specs[i]))
                    for i, a in enumerate(args)]
        zspec = NamedSharding(mesh, PartitionSpec("core"))
        zss = [[jax.device_put(z, zspec) for z in fresh_zeros()]
               for _ in range(k)]
        out = sharded(*dev_args, *zss[0])   # warm
        jax.block_until_ready(out)
        zss = zss[1:]
        jax.block_until_ready(zss)
        t0 = time.perf_counter()
        outs = []
        for zs in zss:
            outs.append(sharded(*dev_args, *zs))
        jax.block_until_ready(outs)
        total = time.perf_counter() - t0
        return unpack(outs[-1]), total, len(zss)

    def run_timed(in_maps, iters=10):
        """Device-resident inputs; returns (results, per-iter seconds list)."""
        import time
        from jax.sharding import NamedSharding
        args = pack(in_maps)
        dev_args = [
            jax.device_put(a, NamedSharding(
                mesh, in_specs[i]))
            for i, a in enumerate(args)
        ]
        out_arrs = sharded(*dev_args, *fresh_zeros())  # warm compile/caches
        jax.block_until_ready(out_arrs)
        times = []
        zspec = NamedSharding(mesh, PartitionSpec("core"))
        for _ in range(iters):
            try:
                zs = [jax.device_put(z, zspec) for z in fresh_zeros()]
                jax.block_until_ready(zs)
                t0 = time.perf_counter()
                out_arrs = sharded(*dev_args, *zs)
                jax.block_until_ready(out_arrs)
                times.append(time.perf_counter() - t0)
            except Exception as exc:  # device hiccup: keep what we have
                print(f"timed iter failed: {exc}", file=sys.stderr)
                break
        return unpack(out_arrs), times

    run.timed = run_timed
    run.timed_pipeline = timed_pipeline
    run.sharded = sharded
    run.pack = pack
    run.in_specs = in_specs
    run.mesh = mesh
    run.fresh_zeros = fresh_zeros
    return run


def _bf16(a):
    import ml_dtypes
    return np.asarray(np.asarray(a, np.float32), dtype=ml_dtypes.bfloat16)


def _f8(a):
    return np.asarray(np.asarray(a, np.float32), dtype=mybir.dt.np(F8))


def _pair_pack_cols(w):
    """(D, D) -> (NPAIR, P, FC, P): per-pair lhsT blocks of interleaved heads."""
    wr = np.asarray(w, np.float32).reshape(D, HD, NH)
    out = np.empty((NPAIR, P, FC, P), np.float32)
    for p in range(NPAIR):
        blk = np.concatenate([wr[:, :, 2 * p], wr[:, :, 2 * p + 1]], axis=1)
        out[p] = blk.reshape(FC, P, P).transpose(1, 0, 2)
    return out


def _pair_pack_rows(w):
    """(D, D) -> (NPAIR, P, D): wo rows grouped by pair (interleaved rows)."""
    wr = np.asarray(w, np.float32).reshape(HD, NH, D)
    out = np.empty((NPAIR, P, D), np.float32)
    for p in range(NPAIR):
        out[p] = np.concatenate([wr[:, 2 * p, :], wr[:, 2 * p + 1, :]], axis=0)
    return out


BIGNEG = 1024.0


def _core_maskAB(blocks):
    """Rank-structured additive masks: scores += A[j].T @ B[j] adds -BIGNEG
    to masked (m, l) pairs of slot j's first suffix l-block; see module doc."""
    S = set(blocks)
    A = np.zeros((MMC, P, P), np.float32)
    Bm = np.zeros((MMC, P, P), np.float32)
    # tri pattern: masked iff m > l  ->  A[k, m] = [m == k+1], B[k, l] = [k >= l]
    a_tri = np.zeros((P, P), np.float32)
    a_tri[np.arange(P - 1), np.arange(1, P)] = 1.0
    b_tri = -BIGNEG * (np.arange(P)[:, None] >= np.arange(P)[None, :])
    for j in range(MMC):
        # local index of first owned block >= j  (4 => no valid l-block)
        idx = next((i for i, b in enumerate(blocks) if b >= j), 4)
        r = idx * P
        s = SUF[j] * P
        if r > s:                     # program slack: kill the dead l-block
            A[j, 0, :] = 1.0
            Bm[j, 0, :] = -BIGNEG
        elif j in S:                  # diagonal block starts the suffix
            A[j] = a_tri
            Bm[j] = b_tri
        # else: fully valid block -> A = B = 0 (adds nothing)
    return A, Bm


def _prepare(inputs):
    x = np.asarray(inputs["x"], np.float32)
    enc = np.asarray(inputs["enc_output"], np.float32)
    smask = np.asarray(inputs["self_attn_mask"])
    cmask = np.asarray(inputs["enc_dec_mask"])

    causal = np.array_equal(
        smask.reshape(L, M), np.triu(np.ones((L, M), bool), k=1))
    crosszero = not cmask.any()
    if not (causal and crosszero):
        return None  # caller falls back to numpy path

    ln_ident = tuple(
        bool(np.all(np.asarray(inputs[f"ln{i}_g"]) == 1.0)
             and np.all(np.asarray(inputs[f"ln{i}_b"]) == 0.0))
        for i in (1, 2, 3))

    sel01 = np.zeros((33, P), np.float32)
    sel01[0, 0:64] = 1.0
    sel01[32, 64:128] = 1.0

    shared = {
        "iddt": _bf16(np.eye(P, dtype=np.float32)),
        "idf32": np.eye(P, dtype=np.float32),
        "sel01": _bf16(sel01),
        "wq_s": _bf16(_pair_pack_cols(inputs["sa_wq"])),
        "wk_s": _bf16(_pair_pack_cols(inputs["sa_wk"])),
        "wo_s": _bf16(_pair_pack_rows(inputs["sa_wo"])),
        "wq_c": _bf16(_pair_pack_cols(inputs["ca_wq"])),
        "wk_c": _bf16(_pair_pack_cols(inputs["ca_wk"])),
        "wo_c": _bf16(_pair_pack_rows(inputs["ca_wo"])),
        "ffw1": _bf16(np.asarray(inputs["ff_w1"], np.float32)
                      .reshape(FC, P, MLPC, P).transpose(2, 1, 0, 3)),
        "ffb1": np.ascontiguousarray(
            np.asarray(inputs["ff_b1"], np.float32).reshape(MLPC, P).T),
        "ffw2": _bf16(np.asarray(inputs["ff_w2"], np.float32).reshape(MLPC, P, D)),
        "ffb2": np.ascontiguousarray(
            np.broadcast_to(np.asarray(inputs["ff_b2"], np.float32), (P, D))),
    }
    for i, ident in enumerate(ln_ident):
        if not ident:
            shared[f"lng{i}"] = np.ascontiguousarray(np.broadcast_to(
                np.asarray(inputs[f"ln{i + 1}_g"], np.float32), (P, D)))
            shared[f"lnb{i}"] = np.ascontiguousarray(np.broadcast_to(
                np.asarray(inputs[f"ln{i + 1}_b"], np.float32), (P, D)))

    in_maps = []
    for c in range(N_CORES):
        b, half = divmod(c, 2)
        blocks = BLK[half]
        mA, mB = _core_maskAB(blocks)
        cols = np.concatenate([np.arange(j * P, (j + 1) * P) for j in blocks])
        xT = x[b].T                                # (D, L)
        in_maps.append(dict(
            shared,
            xkT=_bf16(xT.reshape(FC, P, M)),
            xqT=_bf16(np.ascontiguousarray(xT[:, cols]).reshape(FC, P, LLOC)),
            encT=_bf16(enc[b].T.reshape(FC, P, M)),
            xrows=_bf16(x[b, cols].reshape(LC, P, D)),
            maskA=_bf16(mA), maskB=_bf16(mB),
        ))
    return in_maps, ln_ident


def _numpy_fallback(inputs):
    import scipy.special as sp

    def mha_np(q_in, k_in, mask, wq, wk, wo):
        bq = q_in @ np.asarray(wq, np.float32)
        bk = k_in @ np.asarray(wk, np.float32)
        b_, l_, d_ = bq.shape
        m_ = bk.shape[1]
        q = bq.reshape(b_, l_, HD, NH)
        k = bk.reshape(b_, m_, HD, NH)
        score = np.einsum("bldn,bmdn->blmn", q, k)
        score = np.where(np.asarray(mask), np.float32(-1e9), score)
        score = score / np.float32(HD ** 0.5)
        score = score - score.max(axis=2, keepdims=True)
        e = np.exp(score)
        attn = e / e.sum(axis=2, keepdims=True)
        xx = np.einsum("blmn,bmdn->bldn", attn, k)
        return xx.reshape(b_, l_, d_) @ np.asarray(wo, np.float32)

    def ln(h, g, b):
        mu = h.mean(-1, keepdims=True)
        var = h.var(-1, keepdims=True)
        return (h - mu) / np.sqrt(var + EPS) * np.asarray(g) + np.asarray(b)

    x = np.asarray(inputs["x"], np.float32)
    enc = np.asarray(inputs["enc_output"], np.float32)
    h = x + mha_np(x, x, inputs["self_attn_mask"],
                   inputs["sa_wq"], inputs["sa_wk"], inputs["sa_wo"])
    h = ln(h, inputs["ln1_g"], inputs["ln1_b"])
    h = h + mha_np(h, enc, inputs["enc_dec_mask"],
                   inputs["ca_wq"], inputs["ca_wk"], inputs["ca_wo"])
    h = ln(h, inputs["ln2_g"], inputs["ln2_b"])
    z = (h @ np.asarray(inputs["ff_w1"], np.float32)
         + np.asarray(inputs["ff_b1"], np.float32))
    g = 0.5 * z * (1.0 + sp.erf(z / np.sqrt(2.0)))
    ff = (g @ np.asarray(inputs["ff_w2"], np.float32)
          + np.asarray(inputs["ff_b2"], np.float32))
    h = ln(h + ff, inputs["ln3_g"], inputs["ln3_b"])
    return np.asarray(h, np.float32)


def _get_runner(ln_ident, repeat=1):
    key = (ln_ident, repeat)
    if key not in _CACHE:
        nc = _build_program(ln_ident, repeat=repeat)
        _CACHE[key] = _make_runner(nc, N_CORES)
    return _CACHE[key]


def _assemble(results):
    out = np.empty((B, L, D), np.float32)
    for c in range(N_CORES):
        b, half = divmod(c, 2)
        res = results[c]["out"].reshape(LLOC, D)
        for i, j in enumerate(BLK[half]):
            out[b, j * P:(j + 1) * P] = res[i * P:(i + 1) * P]
    return out


def kernel(**inputs):
    prep = _prepare(inputs)
    if prep is None:
        return _numpy_fallback(inputs)
    in_maps, ln_ident = prep
    run = _get_runner(ln_ident)
    results = run(in_maps)
    return _assemble(results)



# revision 22
# speedup vs baseline: 2.2104x; 2.2104x over previous
"""Trainium2 Bass kernel for nn_DecoderLayer (dense transformer decoder layer).

Sharding: data-parallel over batch (4) x causal-balanced l-block split (2)
= 8 cores, no collectives.  Core (b, half) owns 4 l-blocks of 128 rows:
half 0 -> global blocks {0,3,4,7}, half 1 -> {1,2,5,6}; both sets have equal
causal attention work, served by ONE SPMD program with a uniform per-slot
suffix schedule [4,4,3,3,2,2,1,1] l-blocks and per-core mask data.

v2: fp8(e4m3) DoubleRow matmuls for all projections + FFN (interleaved
accumulation chains across 2 PSUM banks), scores kept bf16 (K=64 tiles
auto-pack in the PE array), exp -> fp8 e feeding DoubleRow attn@k, and a
software-pipelined emission order (projection work of later head-pairs fills
PE gaps left by the scores->exp->attn@k chain of earlier pairs).

Scale folding (zero extra instructions): weights x32 in fp8, attention
psums at 1024x, FFN at 32x; LayerNorm rstd absorbs all rescaling exactly
(LN is scale-invariant; eps constants adjusted per stage).

Faithful to the reference quirks:
  - q/k reshape is (head_dim, n_heads) interleaved -> per-head weight columns
    are strided slices, handled by host-side weight rearrangement.
  - the second attention einsum uses k (not v); v is never computed.
  - mask applied before scaling; softmax without max-subtraction is exact here
    because masked entries are exactly zeroed and scores are O(1).

Softmax row sums come free from the attn@k matmul: each head's ktil block
carries a ones column, putting sum_m e[m,l] in PSUM partition 64.
"""

import sys

sys.path.insert(0, "/opt/trn_rl_repo")

from collections import deque
from contextlib import ExitStack

import numpy as np

import concourse.bass as bass  # noqa: F401  (registers types)
import concourse.mybir as mybir
import concourse.tile as tile
from concourse import bacc

f32 = mybir.dt.float32
DT = mybir.dt.bfloat16
F8 = mybir.dt.float8e4
AF = mybir.ActivationFunctionType
ALU = mybir.AluOpType
DR = mybir.MatmulPerfMode.DoubleRow

P = 128
B, L, D, M = 4, 1024, 1024, 1024
NH, HD, MLP = 16, 64, 4096
NPAIR = NH // 2          # 8 head pairs
FC = D // P              # 8 feature chunks
LLOC = L // 2            # 512 rows per core
LC = LLOC // P           # 4 l-chunks of 128
MMC = M // P             # 8 m-chunks (kv slots)
MLPC = MLP // P          # 32 mlp chunks
EPS = 1e-5
N_CORES = 8

S = 32.0                 # fp8 weight scale
LAM_A = S * S            # attention psum/resid scale (1024)
LAM_F = S                # ffn psum/resid scale (32)
ESC = 0.125 / LAM_A      # exp scale on raw scores
BIGNEG = 1024.0

BLK = ([0, 3, 4, 7], [1, 2, 5, 6])   # l-blocks per core half
# per-slot causal suffix schedule: slot j covers local l cols [SUF[j]*128, 512)
SUF = [0, 0, 1, 1, 2, 2, 3, 3]


def _build_program(ln_ident, repeat=1):
    """ln_ident: tuple of 3 bools -- gamma==1 and beta==0 for each LN."""
    nc = bacc.Bacc(None, target_bir_lowering=False)

    # ---- per-core inputs ----
    xk_d = nc.dram_tensor("xkT", [FC, P, M], F8, kind="ExternalInput")
    xq_d = nc.dram_tensor("xqT", [FC, P, LLOC], F8, kind="ExternalInput")
    enc_d = nc.dram_tensor("encT", [FC, P, M], F8, kind="ExternalInput")
    xr_d = nc.dram_tensor("xrows", [LC, P, D], DT, kind="ExternalInput")
    mka_d = nc.dram_tensor("maskA", [MMC, P, P], DT, kind="ExternalInput")
    mkb_d = nc.dram_tensor("maskB", [MMC, P, P], DT, kind="ExternalInput")
    # ---- shared inputs ----
    iddt_d = nc.dram_tensor("iddt", [P, P], DT, kind="ExternalInput")
    wq_s_d = nc.dram_tensor("wq_s", [NPAIR, P, FC, P], F8, kind="ExternalInput")
    wk_s_d = nc.dram_tensor("wk_s", [NPAIR, P, FC, P], F8, kind="ExternalInput")
    wo_s_d = nc.dram_tensor("wo_s", [NPAIR, P, D], F8, kind="ExternalInput")
    wq_c_d = nc.dram_tensor("wq_c", [NPAIR, P, FC, P], F8, kind="ExternalInput")
    wk_c_d = nc.dram_tensor("wk_c", [NPAIR, P, FC, P], F8, kind="ExternalInput")
    wo_c_d = nc.dram_tensor("wo_c", [NPAIR, P, D], F8, kind="ExternalInput")
    w1_d = nc.dram_tensor("ffw1", [MLPC, P, FC, P], DT, kind="ExternalInput")
    b1_d = nc.dram_tensor("ffb1", [P, MLPC], f32, kind="ExternalInput")
    w2_d = nc.dram_tensor("ffw2", [MLPC, P, D], DT, kind="ExternalInput")
    b2_d = nc.dram_tensor("ffb2", [P, D], f32, kind="ExternalInput")
    ln_bc_d = {}
    for i, ident in enumerate(ln_ident):
        if not ident:
            ln_bc_d[i] = (
                nc.dram_tensor(f"lng{i}", [P, D], f32, kind="ExternalInput"),
                nc.dram_tensor(f"lnb{i}", [P, D], f32, kind="ExternalInput"),
            )
    out_d = nc.dram_tensor("out", [LC, P, D], f32, kind="ExternalOutput")

    # LN rescale constants: (sqrt scale, eps value) per LN stage
    LN_SC = ((1.0 / LAM_A ** 2, EPS),
             (1.0 / LAM_A, EPS * (LAM_A / LAM_F) ** 2),
             (1.0, EPS * LAM_F ** 2))

    with tile.TileContext(nc) as tc:
        with ExitStack() as ctx:
            glob = ctx.enter_context(tc.tile_pool(name="glob", bufs=1))
            psA = ctx.enter_context(tc.tile_pool(name="psA", bufs=2,
                                                 space="PSUM"))

            iddt = glob.tile([P, P], DT)
            nc.gpsimd.dma_start(out=iddt[:], in_=iddt_d[:])
            eps_t = {}
            for i, (_, ev) in enumerate(LN_SC):
                eps_t[i] = glob.tile([P, 1], f32, name=f"eps{i}")
                nc.vector.memset(eps_t[i][:], ev)
            b2bc = glob.tile([P, D], f32)
            nc.scalar.dma_start(out=b2bc[:], in_=b2_d[:])
            b1t = glob.tile([P, MLPC], f32)
            nc.scalar.dma_start(out=b1t[:], in_=b1_d[:])
            ln_bc = {}
            for i, (g_d, b_d) in ln_bc_d.items():
                g_t = glob.tile([P, D], f32, name=f"lng{i}")
                nc.scalar.dma_start(out=g_t[:], in_=g_d[:])
                b_t = glob.tile([P, D], f32, name=f"lnbt{i}")
                nc.scalar.dma_start(out=b_t[:], in_=b_d[:])
                ln_bc[i] = (g_t, b_t)

            # activations / masks loaded once, reused across repeats
            xk = glob.tile([P, FC, M], F8, name="xk")
            nc.gpsimd.dma_start(out=xk[:], in_=xk_d.rearrange("f p m -> p f m"))
            xq = glob.tile([P, FC, LLOC], F8, name="xq")
            nc.gpsimd.dma_start(out=xq[:], in_=xq_d.rearrange("f p m -> p f m"))
            xros = glob.tile([P, LC, D], DT, name="xros")
            nc.gpsimd.dma_start(out=xros[:],
                                in_=xr_d.rearrange("lc li d -> li lc d"))
            maskA = glob.tile([P, MMC, P], DT, name="maskA")
            nc.gpsimd.dma_start(out=maskA[:],
                                in_=mka_d.rearrange("s pi l -> pi s l"))
            maskB = glob.tile([P, MMC, P], DT, name="maskB")
            nc.gpsimd.dma_start(out=maskB[:],
                                in_=mkb_d.rearrange("s pi l -> pi s l"))
            enc = glob.tile([P, FC, M], F8, name="enc")
            nc.gpsimd.dma_start(out=enc[:], in_=enc_d.rearrange("f p m -> p f m"))

            def _ln_finish(pool, h, stats, stage):
                sc, _ = LN_SC[stage]
                mv = pool.tile([P, 2], f32, tag="lnmv", name="lnmv")
                nc.vector.bn_aggr(out=mv[:], in_=stats)
                rstd = pool.tile([P, 1], f32, tag="lnr", name="lnr")
                nc.scalar.activation(out=rstd[:], in_=mv[:, 1:2], func=AF.Sqrt,
                                     bias=eps_t[stage][:], scale=sc)
                nc.vector.reciprocal(out=rstd[:], in_=rstd[:])
                nc.vector.tensor_scalar(
                    out=h[:], in0=h[:], scalar1=mv[:, 0:1], scalar2=rstd[:],
                    op0=ALU.subtract, op1=ALU.mult)
                g_bc, b_bc = ln_bc.get(stage, (None, None))
                if g_bc is not None:
                    nc.vector.tensor_tensor(out=h[:], in0=h[:], in1=g_bc[:],
                                            op=ALU.mult)
                if b_bc is not None:
                    nc.vector.tensor_tensor(out=h[:], in0=h[:], in1=b_bc[:],
                                            op=ALU.add)

            # ---------- global filler queue ----------
            fillers = deque()

            def pump(n=1):
                for _ in range(n):
                    while fillers:
                        try:
                            next(fillers[0])
                            break
                        except StopIteration:
                            fillers.popleft()
                    else:
                        return

            def drain(gen):
                for _ in gen:
                    pass

            for _rep in range(repeat):
              with ExitStack() as rep_ctx:
                work = rep_ctx.enter_context(tc.tile_pool(name="work", bufs=1))
                hpool = rep_ctx.enter_context(tc.tile_pool(name="hp", bufs=1))
                attw_ctx = ExitStack()
                wqk = attw_ctx.enter_context(tc.tile_pool(name="wqk", bufs=1))
                attwork = attw_ctx.enter_context(
                    tc.tile_pool(name="attwork", bufs=1))

                def load_attn_w(wq_d_, wk_d_, wo_d_, sfx):
                    wqa = wqk.tile([P, NPAIR, FC, P], F8, name=f"wqa{sfx}")
                    wka = wqk.tile([P, NPAIR, FC, P], F8, name=f"wka{sfx}")
                    for hv in range(2):
                        nc.sync.dma_start(
                            out=wka[:, hv * 4:(hv + 1) * 4, :, :],
                            in_=wk_d_[hv * 4:(hv + 1) * 4]
                            .rearrange("n p f c -> p n f c"))
                        nc.sync.dma_start(
                            out=wqa[:, hv * 4:(hv + 1) * 4, :, :],
                            in_=wq_d_[hv * 4:(hv + 1) * 4]
                            .rearrange("n p f c -> p n f c"))
                    woa = work.tile([P, NPAIR, D], F8, name=f"woa{sfx}")
                    nc.sync.dma_start(out=woa[:],
                                      in_=wo_d_.rearrange("n p d -> p n d"))
                    return wqa, wka, woa

                wqa_s, wka_s, woa_s = load_attn_w(wq_s_d, wk_s_d, wo_s_d, "s")
                wqa_c, wka_c, woa_c = load_attn_w(wq_c_d, wk_c_d, wo_c_d, "c")

                # ktil layout: [m, slot, head, 64 data | 1 ones | 63 zero]
                KT_BUFS = 8

                def new_kt():
                    return attwork.tile([P, M], DT, tag="kt", bufs=KT_BUFS,
                                     name="kt")

                def new_ktil():
                    kt_i = attwork.tile([P, MMC, 2, P], F8, tag="ktil",
                                     bufs=KT_BUFS, name="ktil")
                    nc.gpsimd.memset(kt_i[:, :, :, 64:128], 0.0)
                    nc.gpsimd.memset(kt_i[:, :, :, 64:65], 1.0)
                    return kt_i

                # ---------- phase 1: projections for one pair ----------
                def ph1_k(p, wka_, kv, slots):
                    """kproj + transposes -> kt, ktil.  kv: [P, FC, M] fp8."""
                    kt = slots["kt"][p] = new_kt()
                    ktil = slots["ktil"][p] = new_ktil()
                    psk = [psA.tile([P, 512], f32, tag="pa", name="psk")
                           for _ in range(2)]
                    for f2 in range(2):
                        for h in range(2):
                            nc.tensor.matmul(
                                psk[h][:], wka_[:, p, 2 * f2:2 * f2 + 2, :],
                                kv[:, 2 * f2:2 * f2 + 2, h * 512:(h + 1) * 512],
                                start=(f2 == 0), stop=False, perf_mode=DR)
                    yield
                    for f2 in range(2, 4):
                        for h in range(2):
                            nc.tensor.matmul(
                                psk[h][:], wka_[:, p, 2 * f2:2 * f2 + 2, :],
                                kv[:, 2 * f2:2 * f2 + 2, h * 512:(h + 1) * 512],
                                start=False, stop=(f2 == 3), perf_mode=DR)
                    for h in range(2):
                        nc.vector.tensor_copy(
                            out=kt[:, h * 512:(h + 1) * 512], in_=psk[h][:])
                    yield
                    pst = psA.tile([P, 2, 512], DT, tag="pa", name="pst")
                    for g in range(2):
                        for m4 in range(4):
                            slot = g * 4 + m4
                            nc.tensor.transpose(
                                pst[:, g, m4 * P:(m4 + 1) * P],
                                kt[:, slot * P:(slot + 1) * P], iddt[:])
                        if g == 0:
                            nc.vector.tensor_copy(
                                out=ktil[:, 0:4, :, 0:64], in_=pst[:, 0, :])
                        else:
                            nc.scalar.copy(
                                out=ktil[:, 4:8, :, 0:64], in_=pst[:, 1, :])
                        yield

                def ph1_q(p, wqa_, qv, slots):
                    """qproj -> qt.  qv: [P, FC, LLOC] fp8."""
                    qt = slots["qt"][p] = attwork.tile([P, LLOC], DT, tag="qt",
                                                    bufs=KT_BUFS, name="qt")
                    psq = psA.tile([P, 512], f32, tag="pa", name="psq")
                    for f2 in range(4):
                        nc.tensor.matmul(
                            psq[:], wqa_[:, p, 2 * f2:2 * f2 + 2, :],
                            qv[:, 2 * f2:2 * f2 + 2, :],
                            start=(f2 == 0), stop=(f2 == 3), perf_mode=DR)
                    nc.vector.tensor_copy(out=qt[:], in_=psq[:])
                    yield

                # ---------- phase 2: scores/softmax/attn@k for one pair ----
                def ph2(p, slots, attno, use_mask, psS, psO):
                    kt, ktil, qt = (slots["kt"][p], slots["ktil"][p],
                                    slots["qt"][p])
                    pso = [psO.tile([P, LLOC], f32, tag=f"po{hi}", name="pso")
                           for hi in range(2)]
                    for g in range(4):
                        e_sg = attwork.tile([P, 2, 2, LLOC], F8, tag="e", bufs=5,
                                         name="e_sg")
                        s0 = SUF[2 * g] * P if use_mask else 0
                        for par in range(2):
                            slot = 2 * g + par
                            pss = psS.tile([P, 2, 512], f32, tag="ps",
                                           name="pss")
                            for hi in range(2):
                                nc.tensor.matmul(
                                    pss[:, hi, s0:],
                                    kt[hi * 64:hi * 64 + 64,
                                       slot * P:(slot + 1) * P],
                                    qt[hi * 64:hi * 64 + 64, s0:],
                                    start=True, stop=not use_mask)
                            if use_mask:
                                for hi in range(2):
                                    nc.tensor.matmul(
                                        pss[:, hi, s0:s0 + P],
                                        maskA[:, slot, :], maskB[:, slot, :],
                                        start=False, stop=True,
                                        skip_group_check=True)
                            yield
                            nc.scalar.activation(
                                out=e_sg[:, :, par, s0:], in_=pss[:, :, s0:],
                                func=AF.Exp, scale=ESC)
                        for hi in range(2):
                            nc.tensor.matmul(
                                pso[hi][:, s0:],
                                ktil[:, 2 * g:2 * g + 2, hi, :],
                                e_sg[:, hi, :, s0:],
                                start=(g == 0), stop=(g == 3), perf_mode=DR)
                        yield
                    # normalize: row sums sit at PSUM partition 64
                    for hi in range(2):
                        sr = work.tile([P, LLOC], DT, tag="sr", bufs=2,
                                       name="sr")
                        with nc.allow_low_precision(reason="softmax denom"):
                            nc.vector.reciprocal(out=sr[0:1, :],
                                                 in_=pso[hi][64:65, :])
                        bc = work.tile([P, LLOC], DT, tag="bc", bufs=2,
                                       name="bc")
                        nc.gpsimd.partition_broadcast(
                            out_ap=bc[0:64, :], in_ap=sr[0:1, :], channels=64)
                        nc.vector.tensor_tensor(
                            out=attno[hi * 64:hi * 64 + 64, p, :],
                            in0=pso[hi][0:64, :], in1=bc[0:64, :], op=ALU.mult)
                    yield

                # ---------- out-proj + residual + LN + transposed copy ----
                def outproj(attno, woa_, resid, h_tiles, hT, stage, cvt, psC):
                    lnst = hpool.tile([P, LC, 2, 6], f32, tag=f"lnst{stage}",
                                      name="lnst")
                    for ng in range(2):
                        psy = [psC.tile([P, 512], f32, tag=f"py{lc}",
                                        name=f"psy{lc}") for lc in range(LC)]
                        for pp in range(4):
                            for lc in range(LC):
                                nc.tensor.matmul(
                                    psy[lc][:],
                                    attno[:, 2 * pp:2 * pp + 2,
                                          lc * P:(lc + 1) * P],
                                    woa_[:, 2 * pp:2 * pp + 2,
                                         ng * 512:(ng + 1) * 512],
                                    start=(pp == 0), stop=(pp == 3),
                                    perf_mode=DR)
                            yield
                        for lc in range(LC):
                            nc.vector.tensor_tensor(
                                out=h_tiles[lc][:, ng * 512:(ng + 1) * 512],
                                in0=psy[lc][:],
                                in1=resid(lc)[:, ng * 512:(ng + 1) * 512],
                                op=ALU.add)
                            nc.vector.bn_stats(
                                out=lnst[:, lc, ng, :],
                                in_=h_tiles[lc][:, ng * 512:(ng + 1) * 512])
                        yield
                    for lc in range(LC):
                        h_t = h_tiles[lc]
                        _ln_finish(hpool, h_t[:], lnst[:, lc, :, :], stage)
                        hc = work.tile([P, D], DT, tag="hc", bufs=2, name="hc")
                        nc.vector.tensor_scalar_mul(hc[:], h_t[:], cvt)
                        yield
                        pst2 = psA.tile([P, 2, 512], DT, tag="pa",
                                        name="pst2")
                        for g in range(2):
                            for f4 in range(4):
                                fo = g * 4 + f4
                                nc.tensor.transpose(
                                    pst2[:, g, f4 * P:(f4 + 1) * P],
                                    hc[:, fo * P:(fo + 1) * P], iddt[:])
                            nc.vector.tensor_copy(
                                out=hT[:, g * 4:(g + 1) * 4,
                                       lc * P:(lc + 1) * P],
                                in_=pst2[:, g, :])
                        yield

                # ================= emission =================
                slots_s = {"kt": {}, "ktil": {}, "qt": {}}
                slots_c = {"kt": {}, "ktil": {}, "qt": {}}
                attno_s = work.tile([P, NPAIR, LLOC], F8, name="attno")
                attno_c = attno_s
                h1 = [hpool.tile([P, D], DT, name=f"h1_{lc}")
                      for lc in range(LC)]
                h2 = [hpool.tile([P, D], DT, name=f"h2_{lc}")
                      for lc in range(LC)]
                h1T = hpool.tile([P, FC, LLOC], F8, name="h1T")
                h2T = hpool.tile([P, FC, LLOC], DT, name="h2T")

                # self attention
                drain(ph1_k(0, wka_s, xk, slots_s))
                drain(ph1_q(0, wqa_s, xq, slots_s))
                drain(ph1_k(1, wka_s, xk, slots_s))
                drain(ph1_q(1, wqa_s, xq, slots_s))
                for p in range(2, NPAIR):
                    fillers.append(ph1_k(p, wka_s, xk, slots_s))
                    fillers.append(ph1_q(p, wqa_s, xq, slots_s))
                for p in range(NPAIR):
                    fillers.append(ph1_k(p, wka_c, enc, slots_c))

                with tc.tile_pool(name="psS", bufs=2, space="PSUM") as psS, \
                     tc.tile_pool(name="psO", bufs=1, space="PSUM") as psO:
                    for p in range(NPAIR):
                        cnt = 0
                        for _ in ph2(p, slots_s, attno_s, True, psS, psO):
                            cnt += 1
                            if cnt % 2 == 0:
                                pump()
                with tc.tile_pool(name="psC", bufs=1, space="PSUM") as psC:
                    for _ in outproj(attno_s, woa_s, lambda lc: xros[:, lc, :],
                                     h1, h1T, 0, 1.0 / LAM_A, psC):
                        pump()

                # cross attention
                drain(ph1_q(0, wqa_c, h1T, slots_c))
                drain(ph1_q(1, wqa_c, h1T, slots_c))
                for p in range(2, NPAIR):
                    fillers.append(ph1_q(p, wqa_c, h1T, slots_c))

                with tc.tile_pool(name="psS2", bufs=2, space="PSUM") as psS, \
                     tc.tile_pool(name="psO2", bufs=1, space="PSUM") as psO:
                    for p in range(NPAIR):
                        cnt = 0
                        for _ in ph2(p, slots_c, attno_c, False, psS, psO):
                            cnt += 1
                            if cnt % 2 == 0:
                                pump()
                # attention projection workspaces no longer needed
                attw_ctx.close()
                fw = rep_ctx.enter_context(tc.tile_pool(name="fw", bufs=1))
                def _w1q_load(q):
                    t = fw.tile([P, 8, FC, P], DT, tag="w1q", bufs=2,
                                name=f"w1q{q}")
                    eng = nc.sync if q % 2 == 0 else nc.gpsimd
                    eng.dma_start(
                        out=t[:], in_=w1_d[q * 8:(q + 1) * 8]
                        .rearrange("m p f c -> p m f c"))
                    return t
                w1q = [_w1q_load(0), _w1q_load(1), None, None]
                with tc.tile_pool(name="psC2", bufs=1, space="PSUM") as psC:
                    cnt = 0
                    for _ in outproj(attno_c, woa_c, lambda lc: h1[lc][:],
                                     h2, h2T, 1, 1.0 / LAM_F, psC):
                        cnt += 1
                        pump()
                pump(64)   # drain any leftover fillers

                # ================= FFN (bf16) =================
                gt = fw.tile([P, MLPC, LLOC], DT, name="gt")
                lnst3 = hpool.tile([P, LC, 4, 6], f32, name="lnst3")
                h3 = [hpool.tile([P, D], f32, name=f"h3_{lc}")
                      for lc in range(LC)]

                # w1: z = h2T @ w1 (x32), gelu -> gt (true scale, bf16)
                with tc.tile_pool(name="psG", bufs=4, space="PSUM") as psG:
                  for q in range(4):
                    if q < 2:
                        w1q[q + 2] = _w1q_load(q + 2)
                    for mcp in range(4):
                        psg = [psG.tile([P, 512], f32, tag="pg", name="psg")
                               for _ in range(2)]
                        for fo in range(FC):
                            for j in range(2):
                                nc.tensor.matmul(
                                    psg[j][:],
                                    w1q[q][:, 2 * mcp + j, fo, :],
                                    h2T[:, fo, :],
                                    start=(fo == 0), stop=(fo == FC - 1))
                        for j in range(2):
                            mc = q * 8 + 2 * mcp + j
                            nc.scalar.activation(
                                out=gt[:, mc, :], in_=psg[j][:],
                                func=AF.Gelu,
                                bias=b1t[:, mc:mc + 1],
                                scale=1.0 / S)

                # w2 per output-quarter, 4 l-chunk chains interleaved
                def _w2q_load(oq):
                    t = fw.tile([P, MLPC, 256], DT, tag="w2q", bufs=2,
                                name=f"w2q{oq}")
                    eng = nc.sync if oq % 2 == 0 else nc.gpsimd
                    eng.dma_start(
                        out=t[:], in_=w2_d[:, :, oq * 256:(oq + 1) * 256]
                        .rearrange("m p d -> p m d"))
                    return t
                w2q = [_w2q_load(0), _w2q_load(1), None, None]
                with tc.tile_pool(name="psY", bufs=1, space="PSUM") as psY:
                    for oq in range(4):
                        if oq < 2:
                            w2q[oq + 2] = _w2q_load(oq + 2)
                        psy2 = [psY.tile([P, 256], f32, tag=f"y{lc}",
                                         name=f"psy2_{lc}") for lc in range(LC)]
                        for mc in range(MLPC):
                            for lc in range(LC):
                                nc.tensor.matmul(
                                    psy2[lc][:],
                                    gt[:, mc, lc * P:(lc + 1) * P],
                                    w2q[oq][:, mc, :],
                                    start=(mc == 0), stop=(mc == MLPC - 1))
                        for lc in range(LC):
                            sl = slice(oq * 256, (oq + 1) * 256)
                            nc.vector.tensor_tensor(
                                out=h3[lc][:, sl], in0=psy2[lc][:],
                                in1=h2[lc][:, sl], op=ALU.add)
                            nc.gpsimd.tensor_tensor(
                                out=h3[lc][:, sl], in0=h3[lc][:, sl],
                                in1=b2bc[:, sl], op=ALU.add)
                            nc.vector.bn_stats(out=lnst3[:, lc, oq, :],
                                               in_=h3[lc][:, sl])
                        if oq == 3:
                            for lc in range(LC):
                                _ln_finish(hpool, h3[lc][:],
                                           lnst3[:, lc, :, :], 2)
                                nc.sync.dma_start(out=out_d[lc], in_=h3[lc][:])

    nc.finalize()
    return nc


# ---------------------------------------------------------------------------
# host side
# ---------------------------------------------------------------------------

_CACHE = {}


def _make_runner(nc, n_cores):
    import jax
    from jax.experimental.shard_map import shard_map
    from jax.sharding import Mesh, PartitionSpec
    from concourse.bass2jax import (_bass_exec_p, install_neuronx_cc_hook,
                                    partition_id_tensor)

    install_neuronx_cc_hook()
    partition_name = (nc.partition_id_tensor.name
                      if nc.partition_id_tensor else None)
    in_names, out_names, out_avals = [], [], []
    for alloc in nc.m.functions[0].allocations:
        if not isinstance(alloc, mybir.MemoryLocationSet):
            continue
        name = alloc.memorylocations[0].name
        if alloc.kind == "ExternalInput":
            if name != partition_name:
                in_names.append(name)
        elif alloc.kind == "ExternalOutput":
            out_names.append(name)
            out_avals.append(jax.core.ShapedArray(tuple(alloc.tensor_shape),
                                                  mybir.dt.np(alloc.dtype)))
    n_params = len(in_names)
    all_names = list(in_names) + list(out_names)
    if partition_name is not None:
        all_names.append(partition_name)

    def _body(*args):
        operands = list(args)
        if partition_name is not None:
            operands.append(partition_id_tensor())
        outs = _bass_exec_p.bind(
            *operands, out_avals=tuple(out_avals), in_names=tuple(all_names),
            out_names=tuple(out_names), lowering_input_output_aliases=(),
            sim_require_finite=True, sim_require_nnan=True, nc=nc)
        return tuple(outs)

    devices = jax.devices()[:n_cores]
    mesh = Mesh(np.asarray(devices), ("core",))
    n_outs = len(out_names)
    donate = tuple(range(n_params, n_params + n_outs))
    # inputs identical on every core are passed replicated (one transfer)
    per_core_names = {"xkT", "xqT", "encT", "xrows", "maskA", "maskB"}
    in_specs = tuple(
        PartitionSpec("core") if name in per_core_names else PartitionSpec()
        for name in in_names
    ) + (PartitionSpec("core"),) * n_outs
    sharded = jax.jit(
        shard_map(_body, mesh=mesh, in_specs=in_specs,
                  out_specs=(PartitionSpec("core"),) * n_outs,
                  check_rep=False),
        donate_argnums=donate, keep_unused=True)

    def pack(in_maps):
        args = []
        for name in in_names:
            if name in per_core_names:
                args.append(np.concatenate(
                    [np.asarray(in_maps[c][name]) for c in range(n_cores)],
                    axis=0))
            else:
                args.append(np.asarray(in_maps[0][name]))
        return args

    def unpack(out_arrs):
        out_arrs = [np.asarray(a) for a in out_arrs]
        return [
            {name: out_arrs[i].reshape(n_cores, *out_avals[i].shape)[c]
             for i, name in enumerate(out_names)}
            for c in range(n_cores)
        ]

    def fresh_zeros():
        return [np.zeros((n_cores * av.shape[0], *av.shape[1:]), av.dtype)
                for av in out_avals]

    def run(in_maps):
        out_arrs = sharded(*pack(in_maps), *fresh_zeros())
        return unpack(out_arrs)

    def timed_pipeline(in_maps, k=16):
        """Issue k executions asynchronously, block once; returns
        (results, total_seconds, k)."""
        import time
        from jax.sharding import NamedSharding
        args = pack(in_maps)
        dev_args = [jax.device_put(a, NamedSharding(mesh, in_specs[i]))
                    for i, a in enumerate(args)]
        zspec = NamedSharding(mesh, PartitionSpec("core"))
        zss = [[jax.device_put(z, zspec) for z in fresh_zeros()]
               for _ in range(k)]
        out = sharded(*dev_args, *zss[0])   # warm
        jax.block_until_ready(out)
        zss = zss[1:]
        jax.block_until_ready(zss)
        t0 = time.perf_counter()
        outs = []
        for zs in zss:
            outs.append(sharded(*dev_args, *zs))
        jax.block_until_ready(outs)
        total = time.perf_counter() - t0
        return unpack(outs[-1]), total, len(zss)

    def run_timed(in_maps, iters=10):
        """Device-resident inputs; returns (results, per-iter seconds list)."""
        import time
        from jax.sharding import NamedSharding
        args = pack(in_maps)
        dev_args = [
            jax.device_put(a, NamedSharding(
                mesh, in_specs[i]))
            for i, a in enumerate(args)
        ]
        out_arrs = sharded(*dev_args, *fresh_zeros())  # warm compile/caches
        jax.block_until_ready(out_arrs)
        times = []
        zspec = NamedSharding(mesh, PartitionSpec("core"))
        for _ in range(iters):
            try:
                zs = [jax.device_put(z, zspec) for z in fresh_zeros()]
                jax.block_until_ready(zs)
                t0 = time.perf_counter()
                out_arrs = sharded(*dev_args, *zs)
                jax.block_until_ready(out_arrs)
                times.append(time.perf_counter() - t0)
            except Exception as exc:  # device hiccup: keep what we have
                print(f"timed iter failed: {exc}", file=sys.stderr)
                break
        return unpack(out_arrs), times

    run.timed = run_timed
    run.timed_pipeline = timed_pipeline
    run.sharded = sharded
    run.pack = pack
    run.in_specs = in_specs
    run.mesh = mesh
    run.fresh_zeros = fresh_zeros
    return run


def _bf16(a):
    import ml_dtypes
    return np.asarray(np.asarray(a, np.float32), dtype=ml_dtypes.bfloat16)


def _f8(a):
    import ml_dtypes
    return np.asarray(np.asarray(a, np.float32), dtype=ml_dtypes.float8_e4m3)


def _pair_pack_cols(w):
    """(D, D) -> (NPAIR, P, FC, P): per-pair lhsT blocks of interleaved heads."""
    wr = np.asarray(w, np.float32).reshape(D, HD, NH)
    out = np.empty((NPAIR, P, FC, P), np.float32)
    for p in range(NPAIR):
        blk = np.concatenate([wr[:, :, 2 * p], wr[:, :, 2 * p + 1]], axis=1)
        out[p] = blk.reshape(FC, P, P).transpose(1, 0, 2)
    return out


def _pair_pack_rows(w):
    """(D, D) -> (NPAIR, P, D): wo rows grouped by pair (interleaved rows)."""
    wr = np.asarray(w, np.float32).reshape(HD, NH, D)
    out = np.empty((NPAIR, P, D), np.float32)
    for p in range(NPAIR):
        out[p] = np.concatenate([wr[:, 2 * p, :], wr[:, 2 * p + 1, :]], axis=0)
    return out


def _core_maskAB(blocks):
    """Rank-structured additive masks: scores += A[j].T @ B[j] adds
    -BIGNEG*LAM_A to masked (m, l) pairs of slot j's first suffix l-block."""
    S_ = set(blocks)
    A = np.zeros((MMC, P, P), np.float32)
    Bm = np.zeros((MMC, P, P), np.float32)
    # tri pattern: masked iff m > l  ->  A[k, m] = [m == k+1], B[k, l] = [k >= l]
    a_tri = np.zeros((P, P), np.float32)
    a_tri[np.arange(P - 1), np.arange(1, P)] = 1.0
    b_tri = -BIGNEG * LAM_A * (np.arange(P)[:, None] >= np.arange(P)[None, :])
    for j in range(MMC):
        # local index of first owned block >= j  (4 => no valid l-block)
        idx = next((i for i, b in enumerate(blocks) if b >= j), 4)
        r = idx * P
        s = SUF[j] * P
        if r > s:                     # program slack: kill the dead l-block
            A[j, 0, :] = 1.0
            Bm[j, 0, :] = -BIGNEG * LAM_A
        elif j in S_:                 # diagonal block starts the suffix
            A[j] = a_tri
            Bm[j] = b_tri
        # else: fully valid block -> A = B = 0 (adds nothing)
    return A, Bm


def _prepare(inputs):
    x = np.asarray(inputs["x"], np.float32)
    enc = np.asarray(inputs["enc_output"], np.float32)
    smask = np.asarray(inputs["self_attn_mask"])
    cmask = np.asarray(inputs["enc_dec_mask"])

    causal = np.array_equal(
        smask.reshape(L, M), np.triu(np.ones((L, M), bool), k=1))
    crosszero = not cmask.any()
    if not (causal and crosszero):
        return None  # caller falls back to numpy path

    ln_ident = tuple(
        bool(np.all(np.asarray(inputs[f"ln{i}_g"]) == 1.0)
             and np.all(np.asarray(inputs[f"ln{i}_b"]) == 0.0))
        for i in (1, 2, 3))

    shared = {
        "iddt": _bf16(np.eye(P, dtype=np.float32)),
        "wq_s": _f8(S * _pair_pack_cols(inputs["sa_wq"])),
        "wk_s": _f8(S * _pair_pack_cols(inputs["sa_wk"])),
        "wo_s": _f8(S * _pair_pack_rows(inputs["sa_wo"])),
        "wq_c": _f8(S * _pair_pack_cols(inputs["ca_wq"])),
        "wk_c": _f8(S * _pair_pack_cols(inputs["ca_wk"])),
        "wo_c": _f8(S * _pair_pack_rows(inputs["ca_wo"])),
        "ffw1": _bf16(S * np.asarray(inputs["ff_w1"], np.float32)
                      .reshape(FC, P, MLPC, P).transpose(2, 1, 0, 3)),
        "ffb1": np.ascontiguousarray(
            np.asarray(inputs["ff_b1"], np.float32).reshape(MLPC, P).T),
        "ffw2": _bf16(S * np.asarray(inputs["ff_w2"], np.float32)
                      .reshape(MLPC, P, D)),
        "ffb2": np.ascontiguousarray(LAM_F * np.broadcast_to(
            np.asarray(inputs["ff_b2"], np.float32), (P, D))),
    }
    for i, ident in enumerate(ln_ident):
        if not ident:
            lam_out = (LAM_A, LAM_F, 1.0)[i]
            shared[f"lng{i}"] = np.ascontiguousarray(np.broadcast_to(
                np.asarray(inputs[f"ln{i + 1}_g"], np.float32), (P, D)))
            shared[f"lnb{i}"] = np.ascontiguousarray(lam_out * np.broadcast_to(
                np.asarray(inputs[f"ln{i + 1}_b"], np.float32), (P, D)))

    in_maps = []
    for c in range(N_CORES):
        b, half = divmod(c, 2)
        blocks = BLK[half]
        mA, mB = _core_maskAB(blocks)
        cols = np.concatenate([np.arange(j * P, (j + 1) * P) for j in blocks])
        xT = x[b].T                                # (D, L)
        in_maps.append(dict(
            shared,
            xkT=_f8(xT.reshape(FC, P, M)),
            xqT=_f8(np.ascontiguousarray(xT[:, cols]).reshape(FC, P, LLOC)),
            encT=_f8(enc[b].T.reshape(FC, P, M)),
            xrows=_bf16(LAM_A * x[b, cols].reshape(LC, P, D)),
            maskA=_bf16(mA), maskB=_bf16(mB),
        ))
    return in_maps, ln_ident


def _numpy_fallback(inputs):
    import scipy.special as sp

    def mha_np(q_in, k_in, mask, wq, wk, wo):
        bq = q_in @ np.asarray(wq, np.float32)
        bk = k_in @ np.asarray(wk, np.float32)
        b_, l_, d_ = bq.shape
        m_ = bk.shape[1]
        q = bq.reshape(b_, l_, HD, NH)
        k = bk.reshape(b_, m_, HD, NH)
        score = np.einsum("bldn,bmdn->blmn", q, k)
        score = np.where(np.asarray(mask), np.float32(-1e9), score)
        score = score / np.float32(HD ** 0.5)
        score = score - score.max(axis=2, keepdims=True)
        e = np.exp(score)
        attn = e / e.sum(axis=2, keepdims=True)
        xx = np.einsum("blmn,bmdn->bldn", attn, k)
        return xx.reshape(b_, l_, d_) @ np.asarray(wo, np.float32)

    def ln(h, g, b):
        mu = h.mean(-1, keepdims=True)
        var = h.var(-1, keepdims=True)
        return (h - mu) / np.sqrt(var + EPS) * np.asarray(g) + np.asarray(b)

    x = np.asarray(inputs["x"], np.float32)
    enc = np.asarray(inputs["enc_output"], np.float32)
    h = x + mha_np(x, x, inputs["self_attn_mask"],
                   inputs["sa_wq"], inputs["sa_wk"], inputs["sa_wo"])
    h = ln(h, inputs["ln1_g"], inputs["ln1_b"])
    h = h + mha_np(h, enc, inputs["enc_dec_mask"],
                   inputs["ca_wq"], inputs["ca_wk"], inputs["ca_wo"])
    h = ln(h, inputs["ln2_g"], inputs["ln2_b"])
    z = (h @ np.asarray(inputs["ff_w1"], np.float32)
         + np.asarray(inputs["ff_b1"], np.float32))
    g = 0.5 * z * (1.0 + sp.erf(z / np.sqrt(2.0)))
    ff = (g @ np.asarray(inputs["ff_w2"], np.float32)
          + np.asarray(inputs["ff_b2"], np.float32))
    h = ln(h + ff, inputs["ln3_g"], inputs["ln3_b"])
    return np.asarray(h, np.float32)


def _get_runner(ln_ident, repeat=1):
    key = (ln_ident, repeat)
    if key not in _CACHE:
        nc = _build_program(ln_ident, repeat=repeat)
        _CACHE[key] = _make_runner(nc, N_CORES)
    return _CACHE[key]


def _assemble(results):
    out = np.empty((B, L, D), np.float32)
    for c in range(N_CORES):
        b, half = divmod(c, 2)
        res = results[c]["out"].reshape(LLOC, D)
        for i, j in enumerate(BLK[half]):
            out[b, j * P:(j + 1) * P] = res[i * P:(i + 1) * P]
    return out


def kernel(**inputs):
    prep = _prepare(inputs)
    if prep is None:
        return _numpy_fallback(inputs)
    in_maps, ln_ident = prep
    run = _get_runner(ln_ident)
    results = run(in_maps)
    return _assemble(results)


# revision 23
# speedup vs baseline: 2.7927x; 1.2635x over previous
"""Trainium2 Bass kernel for nn_DecoderLayer (dense transformer decoder layer).

Sharding: data-parallel over batch (4) x causal-balanced l-block split (2)
= 8 cores, no collectives.  Core (b, half) owns 4 l-blocks of 128 rows:
half 0 -> global blocks {0,3,4,7}, half 1 -> {1,2,5,6}; both sets have equal
causal attention work, served by ONE SPMD program with a uniform per-slot
suffix schedule [4,4,3,3,2,2,1,1] l-blocks and per-core mask data.

v2: fp8(e4m3) DoubleRow matmuls for all projections + FFN (interleaved
accumulation chains across 2 PSUM banks), scores kept bf16 (K=64 tiles
auto-pack in the PE array), exp -> fp8 e feeding DoubleRow attn@k, and a
software-pipelined emission order (projection work of later head-pairs fills
PE gaps left by the scores->exp->attn@k chain of earlier pairs).

Scale folding (zero extra instructions): weights x32 in fp8, attention
psums at 1024x, FFN at 32x; LayerNorm rstd absorbs all rescaling exactly
(LN is scale-invariant; eps constants adjusted per stage).

Faithful to the reference quirks:
  - q/k reshape is (head_dim, n_heads) interleaved -> per-head weight columns
    are strided slices, handled by host-side weight rearrangement.
  - the second attention einsum uses k (not v); v is never computed.
  - mask applied before scaling; softmax without max-subtraction is exact here
    because masked entries are exactly zeroed and scores are O(1).

Softmax row sums come free from the attn@k matmul: each head's ktil block
carries a ones column, putting sum_m e[m,l] in PSUM partition 64.
"""

import sys

sys.path.insert(0, "/opt/trn_rl_repo")

from collections import deque
from contextlib import ExitStack

import numpy as np

import concourse.bass as bass  # noqa: F401  (registers types)
import concourse.mybir as mybir
import concourse.tile as tile
from concourse import bacc

f32 = mybir.dt.float32
DT = mybir.dt.bfloat16
F8 = mybir.dt.float8e4
AF = mybir.ActivationFunctionType
ALU = mybir.AluOpType
DR = mybir.MatmulPerfMode.DoubleRow

P = 128
B, L, D, M = 4, 1024, 1024, 1024
NH, HD, MLP = 16, 64, 4096
NPAIR = NH // 2          # 8 head pairs
FC = D // P              # 8 feature chunks
LLOC = L // 2            # 512 rows per core
LC = LLOC // P           # 4 l-chunks of 128
MMC = M // P             # 8 m-chunks (kv slots)
MLPC = MLP // P          # 32 mlp chunks
EPS = 1e-5
N_CORES = 8

S = 32.0                 # fp8 weight scale
LAM_A = S * S            # attention psum/resid scale (1024)
LAM_F = S                # ffn psum/resid scale (32)
ESC = 0.125 / LAM_A      # exp scale on raw scores
BIGNEG = 1024.0

BLK = ([0, 3, 4, 7], [1, 2, 5, 6])   # l-blocks per core half
# per-slot causal suffix schedule: slot j covers local l cols [SUF[j]*128, 512)
SUF = [0, 0, 1, 1, 2, 2, 3, 3]


def _build_program(ln_ident, repeat=1):
    """ln_ident: tuple of 3 bools -- gamma==1 and beta==0 for each LN."""
    nc = bacc.Bacc(None, target_bir_lowering=False)

    # ---- per-core inputs ----
    xk_d = nc.dram_tensor("xkT", [FC, P, M], F8, kind="ExternalInput")
    xq_d = nc.dram_tensor("xqT", [FC, P, LLOC], F8, kind="ExternalInput")
    enc_d = nc.dram_tensor("encT", [FC, P, M], F8, kind="ExternalInput")
    xr_d = nc.dram_tensor("xrows", [LC, P, D], DT, kind="ExternalInput")
    mka_d = nc.dram_tensor("maskA", [MMC, P, P], DT, kind="ExternalInput")
    mkb_d = nc.dram_tensor("maskB", [MMC, P, P], DT, kind="ExternalInput")
    # ---- shared inputs ----
    iddt_d = nc.dram_tensor("iddt", [P, P], DT, kind="ExternalInput")
    wq_s_d = nc.dram_tensor("wq_s", [NPAIR, P, FC, P], F8, kind="ExternalInput")
    wk_s_d = nc.dram_tensor("wk_s", [NPAIR, P, FC, P], F8, kind="ExternalInput")
    wo_s_d = nc.dram_tensor("wo_s", [NPAIR, P, D], F8, kind="ExternalInput")
    wq_c_d = nc.dram_tensor("wq_c", [NPAIR, P, FC, P], F8, kind="ExternalInput")
    wk_c_d = nc.dram_tensor("wk_c", [NPAIR, P, FC, P], F8, kind="ExternalInput")
    wo_c_d = nc.dram_tensor("wo_c", [NPAIR, P, D], F8, kind="ExternalInput")
    w1_d = nc.dram_tensor("ffw1", [MLPC, P, FC, P], DT, kind="ExternalInput")
    b1_d = nc.dram_tensor("ffb1", [P, MLPC], f32, kind="ExternalInput")
    w2_d = nc.dram_tensor("ffw2", [MLPC, P, D], F8, kind="ExternalInput")
    b2_d = nc.dram_tensor("ffb2", [P, D], f32, kind="ExternalInput")
    ln_bc_d = {}
    for i, ident in enumerate(ln_ident):
        if not ident:
            ln_bc_d[i] = (
                nc.dram_tensor(f"lng{i}", [P, D], f32, kind="ExternalInput"),
                nc.dram_tensor(f"lnb{i}", [P, D], f32, kind="ExternalInput"),
            )
    out_d = nc.dram_tensor("out", [LC, P, D], f32, kind="ExternalOutput")

    # LN rescale constants: (sqrt scale, eps value) per LN stage
    LN_SC = ((1.0 / LAM_A ** 2, EPS),
             (1.0 / LAM_A, EPS * (LAM_A / LAM_F) ** 2),
             (1.0, EPS * LAM_F ** 2))

    with tile.TileContext(nc) as tc:
        with ExitStack() as ctx:
            glob = ctx.enter_context(tc.tile_pool(name="glob", bufs=1))
            psA = ctx.enter_context(tc.tile_pool(name="psA", bufs=2,
                                                 space="PSUM"))

            iddt = glob.tile([P, P], DT)
            nc.gpsimd.dma_start(out=iddt[:], in_=iddt_d[:])
            eps_t = {}
            for i, (_, ev) in enumerate(LN_SC):
                eps_t[i] = glob.tile([P, 1], f32, name=f"eps{i}")
                nc.vector.memset(eps_t[i][:], ev)
            b2bc = glob.tile([P, D], f32)
            nc.scalar.dma_start(out=b2bc[:], in_=b2_d[:])
            b1t = glob.tile([P, MLPC], f32)
            nc.scalar.dma_start(out=b1t[:], in_=b1_d[:])
            ln_bc = {}
            for i, (g_d, b_d) in ln_bc_d.items():
                g_t = glob.tile([P, D], f32, name=f"lng{i}")
                nc.scalar.dma_start(out=g_t[:], in_=g_d[:])
                b_t = glob.tile([P, D], f32, name=f"lnbt{i}")
                nc.scalar.dma_start(out=b_t[:], in_=b_d[:])
                ln_bc[i] = (g_t, b_t)

            # activations / masks loaded once, reused across repeats
            xk = glob.tile([P, FC, M], F8, name="xk")
            nc.gpsimd.dma_start(out=xk[:], in_=xk_d.rearrange("f p m -> p f m"))
            xq = glob.tile([P, FC, LLOC], F8, name="xq")
            nc.gpsimd.dma_start(out=xq[:], in_=xq_d.rearrange("f p m -> p f m"))
            xros = glob.tile([P, LC, D], DT, name="xros")
            nc.gpsimd.dma_start(out=xros[:],
                                in_=xr_d.rearrange("lc li d -> li lc d"))
            maskA = glob.tile([P, MMC, P], DT, name="maskA")
            nc.gpsimd.dma_start(out=maskA[:],
                                in_=mka_d.rearrange("s pi l -> pi s l"))
            maskB = glob.tile([P, MMC, P], DT, name="maskB")
            nc.gpsimd.dma_start(out=maskB[:],
                                in_=mkb_d.rearrange("s pi l -> pi s l"))
            enc = glob.tile([P, FC, M], F8, name="enc")
            nc.gpsimd.dma_start(out=enc[:], in_=enc_d.rearrange("f p m -> p f m"))

            def _ln_finish(pool, h, stats, stage):
                sc, _ = LN_SC[stage]
                mv = pool.tile([P, 2], f32, tag="lnmv", name="lnmv")
                nc.vector.bn_aggr(out=mv[:], in_=stats)
                rstd = pool.tile([P, 1], f32, tag="lnr", name="lnr")
                nc.scalar.activation(out=rstd[:], in_=mv[:, 1:2], func=AF.Sqrt,
                                     bias=eps_t[stage][:], scale=sc)
                nc.vector.reciprocal(out=rstd[:], in_=rstd[:])
                nc.vector.tensor_scalar(
                    out=h[:], in0=h[:], scalar1=mv[:, 0:1], scalar2=rstd[:],
                    op0=ALU.subtract, op1=ALU.mult)
                g_bc, b_bc = ln_bc.get(stage, (None, None))
                if g_bc is not None:
                    nc.vector.tensor_tensor(out=h[:], in0=h[:], in1=g_bc[:],
                                            op=ALU.mult)
                if b_bc is not None:
                    nc.vector.tensor_tensor(out=h[:], in0=h[:], in1=b_bc[:],
                                            op=ALU.add)

            # ---------- global filler queue ----------
            fillers = deque()

            def pump(n=1):
                for _ in range(n):
                    while fillers:
                        try:
                            next(fillers[0])
                            break
                        except StopIteration:
                            fillers.popleft()
                    else:
                        return

            def drain(gen):
                for _ in gen:
                    pass

            for _rep in range(repeat):
              with ExitStack() as rep_ctx:
                work = rep_ctx.enter_context(tc.tile_pool(name="work", bufs=1))
                hpool = rep_ctx.enter_context(tc.tile_pool(name="hp", bufs=1))
                attw_ctx = ExitStack()
                wqk = attw_ctx.enter_context(tc.tile_pool(name="wqk", bufs=1))
                attwork = attw_ctx.enter_context(
                    tc.tile_pool(name="attwork", bufs=1))

                def load_attn_w(wq_d_, wk_d_, wo_d_, sfx):
                    wqa = wqk.tile([P, NPAIR, FC, P], F8, name=f"wqa{sfx}")
                    wka = wqk.tile([P, NPAIR, FC, P], F8, name=f"wka{sfx}")
                    for hv in range(2):
                        nc.sync.dma_start(
                            out=wka[:, hv * 4:(hv + 1) * 4, :, :],
                            in_=wk_d_[hv * 4:(hv + 1) * 4]
                            .rearrange("n p f c -> p n f c"))
                        nc.sync.dma_start(
                            out=wqa[:, hv * 4:(hv + 1) * 4, :, :],
                            in_=wq_d_[hv * 4:(hv + 1) * 4]
                            .rearrange("n p f c -> p n f c"))
                    woa = work.tile([P, NPAIR, D], F8, name=f"woa{sfx}")
                    nc.sync.dma_start(out=woa[:],
                                      in_=wo_d_.rearrange("n p d -> p n d"))
                    return wqa, wka, woa

                wqa_s, wka_s, woa_s = load_attn_w(wq_s_d, wk_s_d, wo_s_d, "s")
                wqa_c, wka_c, woa_c = load_attn_w(wq_c_d, wk_c_d, wo_c_d, "c")

                # ktil layout: [m, slot, head, 64 data | 1 ones | 63 zero]
                KT_BUFS = 8

                def new_kt():
                    return attwork.tile([P, M], DT, tag="kt", bufs=KT_BUFS,
                                     name="kt")

                def new_ktil():
                    kt_i = attwork.tile([P, MMC, 2, P], F8, tag="ktil",
                                     bufs=KT_BUFS, name="ktil")
                    nc.gpsimd.memset(kt_i[:, :, :, 64:128], 0.0)
                    nc.gpsimd.memset(kt_i[:, :, :, 64:65], 1.0)
                    return kt_i

                # ---------- phase 1: projections for one pair ----------
                def ph1_k(p, wka_, kv, slots):
                    """kproj + transposes -> kt, ktil.  kv: [P, FC, M] fp8."""
                    kt = slots["kt"][p] = new_kt()
                    ktil = slots["ktil"][p] = new_ktil()
                    psk = [psA.tile([P, 512], f32, tag="pa", name="psk")
                           for _ in range(2)]
                    for f2 in range(2):
                        for h in range(2):
                            nc.tensor.matmul(
                                psk[h][:], wka_[:, p, 2 * f2:2 * f2 + 2, :],
                                kv[:, 2 * f2:2 * f2 + 2, h * 512:(h + 1) * 512],
                                start=(f2 == 0), stop=False, perf_mode=DR)
                    yield
                    for f2 in range(2, 4):
                        for h in range(2):
                            nc.tensor.matmul(
                                psk[h][:], wka_[:, p, 2 * f2:2 * f2 + 2, :],
                                kv[:, 2 * f2:2 * f2 + 2, h * 512:(h + 1) * 512],
                                start=False, stop=(f2 == 3), perf_mode=DR)
                    for h in range(2):
                        nc.vector.tensor_copy(
                            out=kt[:, h * 512:(h + 1) * 512], in_=psk[h][:])
                    yield
                    pst = psA.tile([P, 2, 512], DT, tag="pa", name="pst")
                    for g in range(2):
                        for m4 in range(4):
                            slot = g * 4 + m4
                            nc.tensor.transpose(
                                pst[:, g, m4 * P:(m4 + 1) * P],
                                kt[:, slot * P:(slot + 1) * P], iddt[:])
                        if g == 0:
                            nc.vector.tensor_copy(
                                out=ktil[:, 0:4, :, 0:64], in_=pst[:, 0, :])
                        else:
                            nc.scalar.copy(
                                out=ktil[:, 4:8, :, 0:64], in_=pst[:, 1, :])
                        yield

                def ph1_q(p, wqa_, qv, slots):
                    """qproj -> qt.  qv: [P, FC, LLOC] fp8."""
                    qt = slots["qt"][p] = attwork.tile([P, LLOC], DT, tag="qt",
                                                    bufs=KT_BUFS, name="qt")
                    psq = psA.tile([P, 512], f32, tag="pa", name="psq")
                    for f2 in range(4):
                        nc.tensor.matmul(
                            psq[:], wqa_[:, p, 2 * f2:2 * f2 + 2, :],
                            qv[:, 2 * f2:2 * f2 + 2, :],
                            start=(f2 == 0), stop=(f2 == 3), perf_mode=DR)
                    nc.vector.tensor_copy(out=qt[:], in_=psq[:])
                    yield

                # ---------- phase 2: scores/softmax/attn@k for one pair ----
                def ph2(p, slots, attno, use_mask, psS, psO):
                    kt, ktil, qt = (slots["kt"][p], slots["ktil"][p],
                                    slots["qt"][p])
                    pso = [psO.tile([P, LLOC], f32, tag=f"po{hi}", name="pso")
                           for hi in range(2)]
                    for g in range(4):
                        e_sg = attwork.tile([P, 2, 2, LLOC], F8, tag="e", bufs=5,
                                         name="e_sg")
                        s0 = SUF[2 * g] * P if use_mask else 0
                        for par in range(2):
                            slot = 2 * g + par
                            pss = psS.tile([P, 2, 512], f32, tag="ps",
                                           name="pss")
                            for hi in range(2):
                                nc.tensor.matmul(
                                    pss[:, hi, s0:],
                                    kt[hi * 64:hi * 64 + 64,
                                       slot * P:(slot + 1) * P],
                                    qt[hi * 64:hi * 64 + 64, s0:],
                                    start=True, stop=not use_mask)
                            if use_mask:
                                for hi in range(2):
                                    nc.tensor.matmul(
                                        pss[:, hi, s0:s0 + P],
                                        maskA[:, slot, :], maskB[:, slot, :],
                                        start=False, stop=True,
                                        skip_group_check=True)
                            yield
                            nc.scalar.activation(
                                out=e_sg[:, :, par, s0:], in_=pss[:, :, s0:],
                                func=AF.Exp, scale=ESC)
                        for hi in range(2):
                            nc.tensor.matmul(
                                pso[hi][:, s0:],
                                ktil[:, 2 * g:2 * g + 2, hi, :],
                                e_sg[:, hi, :, s0:],
                                start=(g == 0), stop=(g == 3), perf_mode=DR)
                        yield
                    # normalize: row sums sit at PSUM partition 64
                    for hi in range(2):
                        sr = work.tile([P, LLOC], DT, tag="sr", bufs=2,
                                       name="sr")
                        with nc.allow_low_precision(reason="softmax denom"):
                            nc.vector.reciprocal(out=sr[0:1, :],
                                                 in_=pso[hi][64:65, :])
                        bc = work.tile([P, LLOC], DT, tag="bc", bufs=2,
                                       name="bc")
                        nc.gpsimd.partition_broadcast(
                            out_ap=bc[0:64, :], in_ap=sr[0:1, :], channels=64)
                        nc.vector.tensor_tensor(
                            out=attno[hi * 64:hi * 64 + 64, p, :],
                            in0=pso[hi][0:64, :], in1=bc[0:64, :], op=ALU.mult)
                    yield

                # ---------- out-proj + residual + LN + transposed copy ----
                def outproj(attno, woa_, resid, h_tiles, hT, stage, cvt, psC):
                    lnst = hpool.tile([P, LC, 2, 6], f32, tag=f"lnst{stage}",
                                      name="lnst")
                    for lc in range(LC):
                        psy = [psC.tile([P, 512], f32, tag=f"py{ng}", bufs=2,
                                        name=f"psy{ng}") for ng in range(2)]
                        for pp in range(4):
                            for ng in range(2):
                                nc.tensor.matmul(
                                    psy[ng][:],
                                    attno[:, 2 * pp:2 * pp + 2,
                                          lc * P:(lc + 1) * P],
                                    woa_[:, 2 * pp:2 * pp + 2,
                                         ng * 512:(ng + 1) * 512],
                                    start=(pp == 0), stop=(pp == 3),
                                    perf_mode=DR)
                            if pp % 2 == 1:
                                yield
                        for ng in range(2):
                            nc.vector.tensor_tensor(
                                out=h_tiles[lc][:, ng * 512:(ng + 1) * 512],
                                in0=psy[ng][:],
                                in1=resid(lc)[:, ng * 512:(ng + 1) * 512],
                                op=ALU.add)
                            nc.vector.bn_stats(
                                out=lnst[:, lc, ng, :],
                                in_=h_tiles[lc][:, ng * 512:(ng + 1) * 512])
                        yield
                        h_t = h_tiles[lc]
                        _ln_finish(hpool, h_t[:], lnst[:, lc, :, :], stage)
                        hc = work.tile([P, D], DT, tag="hc", bufs=2, name="hc")
                        nc.vector.tensor_scalar_mul(hc[:], h_t[:], cvt)
                        yield
                        pst2 = psA.tile([P, 2, 512], DT, tag="pa",
                                        name="pst2")
                        for g in range(2):
                            for f4 in range(4):
                                fo = g * 4 + f4
                                nc.tensor.transpose(
                                    pst2[:, g, f4 * P:(f4 + 1) * P],
                                    hc[:, fo * P:(fo + 1) * P], iddt[:])
                            nc.vector.tensor_copy(
                                out=hT[:, g * 4:(g + 1) * 4,
                                       lc * P:(lc + 1) * P],
                                in_=pst2[:, g, :])
                        yield

                # ================= emission =================
                slots_s = {"kt": {}, "ktil": {}, "qt": {}}
                slots_c = {"kt": {}, "ktil": {}, "qt": {}}
                attno_s = work.tile([P, NPAIR, LLOC], F8, name="attno")
                attno_c = attno_s
                h1 = [hpool.tile([P, D], DT, name=f"h1_{lc}")
                      for lc in range(LC)]
                h2 = [hpool.tile([P, D], DT, name=f"h2_{lc}")
                      for lc in range(LC)]
                h1T = hpool.tile([P, FC, LLOC], F8, name="h1T")
                h2T = hpool.tile([P, FC, LLOC], DT, name="h2T")

                # self attention
                drain(ph1_k(0, wka_s, xk, slots_s))
                drain(ph1_q(0, wqa_s, xq, slots_s))
                drain(ph1_k(1, wka_s, xk, slots_s))
                drain(ph1_q(1, wqa_s, xq, slots_s))
                for p in range(2, NPAIR):
                    fillers.append(ph1_k(p, wka_s, xk, slots_s))
                    fillers.append(ph1_q(p, wqa_s, xq, slots_s))
                for p in range(NPAIR):
                    fillers.append(ph1_k(p, wka_c, enc, slots_c))

                with tc.tile_pool(name="psS", bufs=2, space="PSUM") as psS, \
                     tc.tile_pool(name="psO", bufs=1, space="PSUM") as psO:
                    for p in range(NPAIR):
                        cnt = 0
                        for _ in ph2(p, slots_s, attno_s, True, psS, psO):
                            cnt += 1
                            if cnt % 2 == 0:
                                pump()
                with tc.tile_pool(name="psC", bufs=1, space="PSUM") as psC:
                    for _ in outproj(attno_s, woa_s, lambda lc: xros[:, lc, :],
                                     h1, h1T, 0, 1.0 / LAM_A, psC):
                        pump()

                # cross attention
                drain(ph1_q(0, wqa_c, h1T, slots_c))
                drain(ph1_q(1, wqa_c, h1T, slots_c))
                for p in range(2, NPAIR):
                    fillers.append(ph1_q(p, wqa_c, h1T, slots_c))

                with tc.tile_pool(name="psS2", bufs=2, space="PSUM") as psS, \
                     tc.tile_pool(name="psO2", bufs=1, space="PSUM") as psO:
                    for p in range(NPAIR):
                        cnt = 0
                        for _ in ph2(p, slots_c, attno_c, False, psS, psO):
                            cnt += 1
                            if cnt % 2 == 0:
                                pump()
                # attention projection workspaces no longer needed
                attw_ctx.close()
                fw = rep_ctx.enter_context(tc.tile_pool(name="fw", bufs=1))
                def _w1q_load(q):
                    t = fw.tile([P, 8, FC, P], DT, tag="w1q", bufs=2,
                                name=f"w1q{q}")
                    eng = nc.sync if q % 2 == 0 else nc.gpsimd
                    eng.dma_start(
                        out=t[:], in_=w1_d[q * 8:(q + 1) * 8]
                        .rearrange("m p f c -> p m f c"))
                    return t
                w1q = [_w1q_load(0), _w1q_load(1), None, None]
                with tc.tile_pool(name="psC2", bufs=1, space="PSUM") as psC:
                    cnt = 0
                    for _ in outproj(attno_c, woa_c, lambda lc: h1[lc][:],
                                     h2, h2T, 1, 1.0 / LAM_F, psC):
                        cnt += 1
                        pump()
                pump(64)   # drain any leftover fillers

                # ================= FFN (bf16) =================
                gt = fw.tile([P, MLPC, LLOC], F8, name="gt")
                lnst3 = hpool.tile([P, LC, 4, 6], f32, name="lnst3")
                h3 = [hpool.tile([P, D], f32, name=f"h3_{lc}")
                      for lc in range(LC)]

                # w1: z = h2T @ w1 (x32), gelu -> gt (true scale, bf16)
                with tc.tile_pool(name="psG", bufs=4, space="PSUM") as psG:
                  for q in range(4):
                    if q < 2:
                        w1q[q + 2] = _w1q_load(q + 2)
                    for mcp in range(4):
                        psg = [psG.tile([P, 512], f32, tag="pg", name="psg")
                               for _ in range(2)]
                        for fo in range(FC):
                            for j in range(2):
                                nc.tensor.matmul(
                                    psg[j][:],
                                    w1q[q][:, 2 * mcp + j, fo, :],
                                    h2T[:, fo, :],
                                    start=(fo == 0), stop=(fo == FC - 1))
                        for j in range(2):
                            mc = q * 8 + 2 * mcp + j
                            nc.scalar.activation(
                                out=gt[:, mc, :], in_=psg[j][:],
                                func=AF.Gelu,
                                bias=b1t[:, mc:mc + 1],
                                scale=1.0 / S)

                # w2 per output-quarter, 4 l-chunk chains interleaved
                def _w2q_load(oq):
                    t = fw.tile([P, MLPC, 256], F8, tag="w2q", bufs=2,
                                name=f"w2q{oq}")
                    eng = nc.sync if oq % 2 == 0 else nc.gpsimd
                    eng.dma_start(
                        out=t[:], in_=w2_d[:, :, oq * 256:(oq + 1) * 256]
                        .rearrange("m p d -> p m d"))
                    return t
                w2q = [_w2q_load(0), _w2q_load(1), None, None]
                with tc.tile_pool(name="psY", bufs=1, space="PSUM") as psY:
                    for oq in range(4):
                        if oq < 2:
                            w2q[oq + 2] = _w2q_load(oq + 2)
                        psy2 = [psY.tile([P, 256], f32, tag=f"y{lc}",
                                         name=f"psy2_{lc}") for lc in range(LC)]
                        for m2 in range(MLPC // 2):
                            for lc in range(LC):
                                nc.tensor.matmul(
                                    psy2[lc][:],
                                    gt[:, 2 * m2:2 * m2 + 2,
                                       lc * P:(lc + 1) * P],
                                    w2q[oq][:, 2 * m2:2 * m2 + 2, :],
                                    start=(m2 == 0),
                                    stop=(m2 == MLPC // 2 - 1),
                                    perf_mode=DR)
                        for lc in range(LC):
                            sl = slice(oq * 256, (oq + 1) * 256)
                            nc.vector.tensor_tensor(
                                out=h3[lc][:, sl], in0=psy2[lc][:],
                                in1=h2[lc][:, sl], op=ALU.add)
                            nc.gpsimd.tensor_tensor(
                                out=h3[lc][:, sl], in0=h3[lc][:, sl],
                                in1=b2bc[:, sl], op=ALU.add)
                            nc.vector.bn_stats(out=lnst3[:, lc, oq, :],
                                               in_=h3[lc][:, sl])
                        if oq == 3:
                            for lc in range(LC):
                                _ln_finish(hpool, h3[lc][:],
                                           lnst3[:, lc, :, :], 2)
                                nc.sync.dma_start(out=out_d[lc], in_=h3[lc][:])

    nc.finalize()
    return nc


# ---------------------------------------------------------------------------
# host side
# ---------------------------------------------------------------------------

_CACHE = {}


def _make_runner(nc, n_cores):
    import jax
    from jax.experimental.shard_map import shard_map
    from jax.sharding import Mesh, PartitionSpec
    from concourse.bass2jax import (_bass_exec_p, install_neuronx_cc_hook,
                                    partition_id_tensor)

    install_neuronx_cc_hook()
    partition_name = (nc.partition_id_tensor.name
                      if nc.partition_id_tensor else None)
    in_names, out_names, out_avals = [], [], []
    for alloc in nc.m.functions[0].allocations:
        if not isinstance(alloc, mybir.MemoryLocationSet):
            continue
        name = alloc.memorylocations[0].name
        if alloc.kind == "ExternalInput":
            if name != partition_name:
                in_names.append(name)
        elif alloc.kind == "ExternalOutput":
            out_names.append(name)
            out_avals.append(jax.core.ShapedArray(tuple(alloc.tensor_shape),
                                                  mybir.dt.np(alloc.dtype)))
    n_params = len(in_names)
    all_names = list(in_names) + list(out_names)
    if partition_name is not None:
        all_names.append(partition_name)

    def _body(*args):
        operands = list(args)
        if partition_name is not None:
            operands.append(partition_id_tensor())
        outs = _bass_exec_p.bind(
            *operands, out_avals=tuple(out_avals), in_names=tuple(all_names),
            out_names=tuple(out_names), lowering_input_output_aliases=(),
            sim_require_finite=True, sim_require_nnan=True, nc=nc)
        return tuple(outs)

    devices = jax.devices()[:n_cores]
    mesh = Mesh(np.asarray(devices), ("core",))
    n_outs = len(out_names)
    donate = tuple(range(n_params, n_params + n_outs))
    # inputs identical on every core are passed replicated (one transfer)
    per_core_names = {"xkT", "xqT", "encT", "xrows", "maskA", "maskB"}
    in_specs = tuple(
        PartitionSpec("core") if name in per_core_names else PartitionSpec()
        for name in in_names
    ) + (PartitionSpec("core"),) * n_outs
    sharded = jax.jit(
        shard_map(_body, mesh=mesh, in_specs=in_specs,
                  out_specs=(PartitionSpec("core"),) * n_outs,
                  check_rep=False),
        donate_argnums=donate, keep_unused=True)

    def pack(in_maps):
        args = []
        for name in in_names:
            if name in per_core_names:
                args.append(np.concatenate(
                    [np.asarray(in_maps[c][name]) for c in range(n_cores)],
                    axis=0))
            else:
                args.append(np.asarray(in_maps[0][name]))
        return args

    def unpack(out_arrs):
        out_arrs = [np.asarray(a) for a in out_arrs]
        return [
            {name: out_arrs[i].reshape(n_cores, *out_avals[i].shape)[c]
             for i, name in enumerate(out_names)}
            for c in range(n_cores)
        ]

    def fresh_zeros():
        return [np.zeros((n_cores * av.shape[0], *av.shape[1:]), av.dtype)
                for av in out_avals]

    def run(in_maps):
        out_arrs = sharded(*pack(in_maps), *fresh_zeros())
        return unpack(out_arrs)

    def timed_pipeline(in_maps, k=16):
        """Issue k executions asynchronously, block once; returns
        (results, total_seconds, k)."""
        import time
        from jax.sharding import NamedSharding
        args = pack(in_maps)
        dev_args = [jax.device_put(a, NamedSharding(mesh, in_specs[i]))
                    for i, a in enumerate(args)]
        zspec = NamedSharding(mesh, PartitionSpec("core"))
        zss = [[jax.device_put(z, zspec) for z in fresh_zeros()]
               for _ in range(k)]
        out = sharded(*dev_args, *zss[0])   # warm
        jax.block_until_ready(out)
        zss = zss[1:]
        jax.block_until_ready(zss)
        t0 = time.perf_counter()
        outs = []
        for zs in zss:
            outs.append(sharded(*dev_args, *zs))
        jax.block_until_ready(outs)
        total = time.perf_counter() - t0
        return unpack(outs[-1]), total, len(zss)

    def run_timed(in_maps, iters=10):
        """Device-resident inputs; returns (results, per-iter seconds list)."""
        import time
        from jax.sharding import NamedSharding
        args = pack(in_maps)
        dev_args = [
            jax.device_put(a, NamedSharding(
                mesh, in_specs[i]))
            for i, a in enumerate(args)
        ]
        out_arrs = sharded(*dev_args, *fresh_zeros())  # warm compile/caches
        jax.block_until_ready(out_arrs)
        times = []
        zspec = NamedSharding(mesh, PartitionSpec("core"))
        for _ in range(iters):
            try:
                zs = [jax.device_put(z, zspec) for z in fresh_zeros()]
                jax.block_until_ready(zs)
                t0 = time.perf_counter()
                out_arrs = sharded(*dev_args, *zs)
                jax.block_until_ready(out_arrs)
                times.append(time.perf_counter() - t0)
            except Exception as exc:  # device hiccup: keep what we have
                print(f"timed iter failed: {exc}", file=sys.stderr)
                break
        return unpack(out_arrs), times

    run.timed = run_timed
    run.timed_pipeline = timed_pipeline
    run.sharded = sharded
    run.pack = pack
    run.in_specs = in_specs
    run.mesh = mesh
    run.fresh_zeros = fresh_zeros
    return run


def _bf16(a):
    import ml_dtypes
    return np.asarray(np.asarray(a, np.float32), dtype=ml_dtypes.bfloat16)


def _f8(a):
    import ml_dtypes
    return np.asarray(np.asarray(a, np.float32), dtype=ml_dtypes.float8_e4m3)


def _pair_pack_cols(w):
    """(D, D) -> (NPAIR, P, FC, P): per-pair lhsT blocks of interleaved heads."""
    wr = np.asarray(w, np.float32).reshape(D, HD, NH)
    out = np.empty((NPAIR, P, FC, P), np.float32)
    for p in range(NPAIR):
        blk = np.concatenate([wr[:, :, 2 * p], wr[:, :, 2 * p + 1]], axis=1)
        out[p] = blk.reshape(FC, P, P).transpose(1, 0, 2)
    return out


def _pair_pack_rows(w):
    """(D, D) -> (NPAIR, P, D): wo rows grouped by pair (interleaved rows)."""
    wr = np.asarray(w, np.float32).reshape(HD, NH, D)
    out = np.empty((NPAIR, P, D), np.float32)
    for p in range(NPAIR):
        out[p] = np.concatenate([wr[:, 2 * p, :], wr[:, 2 * p + 1, :]], axis=0)
    return out


def _core_maskAB(blocks):
    """Rank-structured additive masks: scores += A[j].T @ B[j] adds
    -BIGNEG*LAM_A to masked (m, l) pairs of slot j's first suffix l-block."""
    S_ = set(blocks)
    A = np.zeros((MMC, P, P), np.float32)
    Bm = np.zeros((MMC, P, P), np.float32)
    # tri pattern: masked iff m > l  ->  A[k, m] = [m == k+1], B[k, l] = [k >= l]
    a_tri = np.zeros((P, P), np.float32)
    a_tri[np.arange(P - 1), np.arange(1, P)] = 1.0
    b_tri = -BIGNEG * LAM_A * (np.arange(P)[:, None] >= np.arange(P)[None, :])
    for j in range(MMC):
        # local index of first owned block >= j  (4 => no valid l-block)
        idx = next((i for i, b in enumerate(blocks) if b >= j), 4)
        r = idx * P
        s = SUF[j] * P
        if r > s:                     # program slack: kill the dead l-block
            A[j, 0, :] = 1.0
            Bm[j, 0, :] = -BIGNEG * LAM_A
        elif j in S_:                 # diagonal block starts the suffix
            A[j] = a_tri
            Bm[j] = b_tri
        # else: fully valid block -> A = B = 0 (adds nothing)
    return A, Bm


def _prepare(inputs):
    x = np.asarray(inputs["x"], np.float32)
    enc = np.asarray(inputs["enc_output"], np.float32)
    smask = np.asarray(inputs["self_attn_mask"])
    cmask = np.asarray(inputs["enc_dec_mask"])

    causal = np.array_equal(
        smask.reshape(L, M), np.triu(np.ones((L, M), bool), k=1))
    crosszero = not cmask.any()
    if not (causal and crosszero):
        return None  # caller falls back to numpy path

    ln_ident = tuple(
        bool(np.all(np.asarray(inputs[f"ln{i}_g"]) == 1.0)
             and np.all(np.asarray(inputs[f"ln{i}_b"]) == 0.0))
        for i in (1, 2, 3))

    shared = {
        "iddt": _bf16(np.eye(P, dtype=np.float32)),
        "wq_s": _f8(S * _pair_pack_cols(inputs["sa_wq"])),
        "wk_s": _f8(S * _pair_pack_cols(inputs["sa_wk"])),
        "wo_s": _f8(S * _pair_pack_rows(inputs["sa_wo"])),
        "wq_c": _f8(S * _pair_pack_cols(inputs["ca_wq"])),
        "wk_c": _f8(S * _pair_pack_cols(inputs["ca_wk"])),
        "wo_c": _f8(S * _pair_pack_rows(inputs["ca_wo"])),
        "ffw1": _bf16(S * np.asarray(inputs["ff_w1"], np.float32)
                      .reshape(FC, P, MLPC, P).transpose(2, 1, 0, 3)),
        "ffb1": np.ascontiguousarray(
            np.asarray(inputs["ff_b1"], np.float32).reshape(MLPC, P).T),
        "ffw2": _f8(S * np.asarray(inputs["ff_w2"], np.float32)
                    .reshape(MLPC, P, D)),
        "ffb2": np.ascontiguousarray(LAM_F * np.broadcast_to(
            np.asarray(inputs["ff_b2"], np.float32), (P, D))),
    }
    for i, ident in enumerate(ln_ident):
        if not ident:
            lam_out = (LAM_A, LAM_F, 1.0)[i]
            shared[f"lng{i}"] = np.ascontiguousarray(np.broadcast_to(
                np.asarray(inputs[f"ln{i + 1}_g"], np.float32), (P, D)))
            shared[f"lnb{i}"] = np.ascontiguousarray(lam_out * np.broadcast_to(
                np.asarray(inputs[f"ln{i + 1}_b"], np.float32), (P, D)))

    in_maps = []
    for c in range(N_CORES):
        b, half = divmod(c, 2)
        blocks = BLK[half]
        mA, mB = _core_maskAB(blocks)
        cols = np.concatenate([np.arange(j * P, (j + 1) * P) for j in blocks])
        xT = x[b].T                                # (D, L)
        in_maps.append(dict(
            shared,
            xkT=_f8(xT.reshape(FC, P, M)),
            xqT=_f8(np.ascontiguousarray(xT[:, cols]).reshape(FC, P, LLOC)),
            encT=_f8(enc[b].T.reshape(FC, P, M)),
            xrows=_bf16(LAM_A * x[b, cols].reshape(LC, P, D)),
            maskA=_bf16(mA), maskB=_bf16(mB),
        ))
    return in_maps, ln_ident


def _numpy_fallback(inputs):
    import scipy.special as sp

    def mha_np(q_in, k_in, mask, wq, wk, wo):
        bq = q_in @ np.asarray(wq, np.float32)
        bk = k_in @ np.asarray(wk, np.float32)
        b_, l_, d_ = bq.shape
        m_ = bk.shape[1]
        q = bq.reshape(b_, l_, HD, NH)
        k = bk.reshape(b_, m_, HD, NH)
        score = np.einsum("bldn,bmdn->blmn", q, k)
        score = np.where(np.asarray(mask), np.float32(-1e9), score)
        score = score / np.float32(HD ** 0.5)
        score = score - score.max(axis=2, keepdims=True)
        e = np.exp(score)
        attn = e / e.sum(axis=2, keepdims=True)
        xx = np.einsum("blmn,bmdn->bldn", attn, k)
        return xx.reshape(b_, l_, d_) @ np.asarray(wo, np.float32)

    def ln(h, g, b):
        mu = h.mean(-1, keepdims=True)
        var = h.var(-1, keepdims=True)
        return (h - mu) / np.sqrt(var + EPS) * np.asarray(g) + np.asarray(b)

    x = np.asarray(inputs["x"], np.float32)
    enc = np.asarray(inputs["enc_output"], np.float32)
    h = x + mha_np(x, x, inputs["self_attn_mask"],
                   inputs["sa_wq"], inputs["sa_wk"], inputs["sa_wo"])
    h = ln(h, inputs["ln1_g"], inputs["ln1_b"])
    h = h + mha_np(h, enc, inputs["enc_dec_mask"],
                   inputs["ca_wq"], inputs["ca_wk"], inputs["ca_wo"])
    h = ln(h, inputs["ln2_g"], inputs["ln2_b"])
    z = (h @ np.asarray(inputs["ff_w1"], np.float32)
         + np.asarray(inputs["ff_b1"], np.float32))
    g = 0.5 * z * (1.0 + sp.erf(z / np.sqrt(2.0)))
    ff = (g @ np.asarray(inputs["ff_w2"], np.float32)
          + np.asarray(inputs["ff_b2"], np.float32))
    h = ln(h + ff, inputs["ln3_g"], inputs["ln3_b"])
    return np.asarray(h, np.float32)


def _get_runner(ln_ident, repeat=1):
    key = (ln_ident, repeat)
    if key not in _CACHE:
        nc = _build_program(ln_ident, repeat=repeat)
        _CACHE[key] = _make_runner(nc, N_CORES)
    return _CACHE[key]


def _assemble(results):
    out = np.empty((B, L, D), np.float32)
    for c in range(N_CORES):
        b, half = divmod(c, 2)
        res = results[c]["out"].reshape(LLOC, D)
        for i, j in enumerate(BLK[half]):
            out[b, j * P:(j + 1) * P] = res[i * P:(i + 1) * P]
    return out


def kernel(**inputs):
    prep = _prepare(inputs)
    if prep is None:
        return _numpy_fallback(inputs)
    in_maps, ln_ident = prep
    run = _get_runner(ln_ident)
    results = run(in_maps)
    return _assemble(results)
